# revision 53
# baseline (speedup 1.0000x reference)
"""Trainium2 Bass kernel for nn_EncoderMixtureModelTrajectory.

Model: 3-layer ReLU MLP (512->512->512->512) -> softmax router over 16
classes (argmax only is needed) -> per-class gaussian expert means
(mu = first 16 of 32 outputs), gather selected class's mu per sample.

Strategy: pure data-parallel over 8 NeuronCores (batch 65536 -> 8192/core).
Feature-major activations [D, N] on-chip; host pre-transposes x and weights.
Precision: fp16 hi/lo split matmuls (3 fp16 matmuls/pair ~ fp32 accuracy)
for the m-chain + router (the argmax decision is precision-critical);
single fp16 for the expert layer and selection machinery.
Router argmax + expert selection are done with small matmuls on the PE
(max-tree on DVE, one-hot via is_equal, first-match via triangular matmul,
select via expand/contract 0-1 matrices, batch-major transpose via identity
matmul).
"""

import os
import sys

if "/opt/trn_rl_repo" not in sys.path:
    sys.path.insert(0, "/opt/trn_rl_repo")
# Recover automatically if a previous run left a NeuronCore wedged.
os.environ.setdefault("NEURON_RT_RESET_CORES", "1")

import numpy as np

# Problem constants (hardcoded per task contract)
B = 65536
D = 512
LAT = 16
K = 16
NCORES = 8
BL = B // NCORES          # 8192 samples per core
TN = 512                  # batch-tile (free dim per matmul)
PC = 128                  # partitions
KC = D // PC              # 4 contraction chunks
MO = D // PC              # 4 output chunks for D-wide layers
MU = K * LAT              # 256 expert-mean rows
MUC = MU // PC            # 2 chunks

_CACHE = {}


def _dt():
    import concourse.mybir as mybir
    return mybir.dt


def build_program(bl=BL, tn=TN, stage=5):
    """Build the Bass/Tile program for one core (SPMD across 8).

    stage: debug bisection level (5 = full kernel)."""
    import concourse.bass as bass
    import concourse.bacc as bacc
    import concourse.mybir as mybir
    import concourse.tile as tile

    dt = mybir.dt
    f16, f32, i32 = dt.float16, dt.float32, dt.int32
    nt = bl // tn

    nc = bacc.Bacc("TRN2", target_bir_lowering=False, debug=False,
                   num_devices=NCORES)

    def din(name, shape, dtype):
        return nc.dram_tensor(name, shape, dtype, kind="ExternalInput").ap()

    def dout(name, shape, dtype):
        return nc.dram_tensor(name, shape, dtype, kind="ExternalOutput").ap()

    xhi_d = din("xhi", [D, bl], f16)
    xlo_d = din("xlo", [D, bl], f16)
    w_d = {}
    for li in range(2):
        for part in ("hi", "lo"):
            w_d[(li, part)] = din(f"w{li}{part}", [D, D], f16)
    wchi_d = din("wchi", [D, K], f16)
    wclo_d = din("wclo", [D, K], f16)
    wgm_d = din("wgm", [D, MU], f16)
    b_d = [din(f"b{li}", [D], f32) for li in range(2)]
    bgm_d = din("bgm", [MU], f32)
    bc_d = din("bc", [K], f32)
    e0_d = din("e0", [K, PC], f16)
    e1_d = din("e1", [K, PC], f16)
    r_d = din("r", [PC, K], f16)
    i16_d = din("i16", [K, K], f16)
    i16f_d = din("i16f", [K, K], f32)
    i128_d = din("i128", [PC, PC], f16)
    idxb_d = din("idxb", [PC, K], f32)
    idxc_d = din("idxc", [PC, K], f32)

    z_d = dout("z", [bl, LAT], f32)
    y_d = dout("y", [bl], i32)

    with tile.TileContext(nc) as tc:
        with (
            tc.tile_pool(name="wpool", bufs=1) as wp,
            tc.tile_pool(name="xpool", bufs=3) as xp,
            tc.tile_pool(name="hpool", bufs=2) as hp,
            tc.tile_pool(name="iopool", bufs=3) as iop,
            tc.tile_pool(name="spool", bufs=3) as sp,
            tc.tile_pool(name="pbig", bufs=4, space="PSUM") as pb,
            tc.tile_pool(name="psmall", bufs=4, space="PSUM") as ps,
        ):
            # ---- resident weights/constants ----
            # w0 (needed first) goes on the sync queue; everything else on
            # the gpsimd queue so tile-0's x/w0 DMAs aren't queued behind it.
            xhi_r = xhi_d.rearrange("(c p) n -> p c n", p=PC)
            xlo_r = xlo_d.rearrange("(c p) n -> p c n", p=PC)
            xhi0 = xp.tile([PC, KC, tn], f16, tag="xhi")
            xlo0 = xp.tile([PC, KC, tn], f16, tag="xlo")
            nc.sync.dma_start(xhi0[:], xhi_r[:, :, 0:tn])
            nc.scalar.dma_start(xlo0[:], xlo_r[:, :, 0:tn])
            w_sb = {}
            for key, d in w_d.items():
                t = wp.tile([PC, KC, D], f16, tag=f"w{key[0]}{key[1]}")
                if key[0] == 0:
                    q = nc.scalar if key[1] == "hi" else nc.sync
                else:
                    q = nc.gpsimd
                q.dma_start(t[:], d.rearrange("(c p) o -> p c o", p=PC))
                w_sb[key] = t
            wchi_sb = wp.tile([PC, KC, K], f16, tag="wchi")
            nc.gpsimd.dma_start(wchi_sb[:], wchi_d.rearrange("(c p) o -> p c o", p=PC))
            wclo_sb = wp.tile([PC, KC, K], f16, tag="wclo")
            nc.gpsimd.dma_start(wclo_sb[:], wclo_d.rearrange("(c p) o -> p c o", p=PC))
            wgm_sb = wp.tile([PC, KC, MU], f16, tag="wgm")
            nc.gpsimd.dma_start(wgm_sb[:], wgm_d.rearrange("(c p) o -> p c o", p=PC))

            b_sb = []
            for li in range(2):
                t = wp.tile([PC, MO], f32, tag=f"b{li}")
                nc.gpsimd.dma_start(t[:], b_d[li].rearrange("(c p) -> p c", p=PC))
                b_sb.append(t)
            bgm_sb = wp.tile([PC, MUC], f32, tag="bgm")
            nc.gpsimd.dma_start(bgm_sb[:], bgm_d.rearrange("(c p) -> p c", p=PC))
            bc_sb = wp.tile([K, 1], f32, tag="bc")
            nc.gpsimd.dma_start(bc_sb[:], bc_d.rearrange("(k o) -> k o", o=1))
            e0_sb = wp.tile([K, PC], f16, tag="e0")
            nc.gpsimd.dma_start(e0_sb[:], e0_d[:])
            e1_sb = wp.tile([K, PC], f16, tag="e1")
            nc.gpsimd.dma_start(e1_sb[:], e1_d[:])
            r_sb = wp.tile([PC, K], f16, tag="r")
            nc.gpsimd.dma_start(r_sb[:], r_d[:])
            i16_sb = wp.tile([K, K], f16, tag="i16")
            nc.gpsimd.dma_start(i16_sb[:], i16_d[:])
            idxb_sb = wp.tile([PC, K], f32, tag="idxb")
            nc.gpsimd.dma_start(idxb_sb[:], idxb_d[:])
            idxc_sb = wp.tile([PC, K], f32, tag="idxc")
            nc.gpsimd.dma_start(idxc_sb[:], idxc_d[:])
            i16f_sb = wp.tile([K, K], f32, tag="i16f")
            nc.gpsimd.dma_start(i16f_sb[:], i16f_d[:])
            i128_sb = wp.tile([PC, PC], f16, tag="i128")
            nc.gpsimd.dma_start(i128_sb[:], i128_d[:])


            def split_layer(rhs_hi, rhs_lo, li, relu):
                """rhs_{hi,lo}: [PC, KC, tn] f16 -> returns (hi, lo) fp16
                tiles [PC, MO, tn] of relu(W x + b).

                hhi comes straight off the ACT engine (fp16 write rounds);
                the fp32 relu and the residual are computed on DVE in
                parallel, so the next layer's hi-term matmuls (issued
                first) only wait one ACT hop."""
                assert relu
                h32 = hp.tile([PC, MO, tn], f32, tag="h32")
                hhi = iop.tile([PC, MO, tn], f16, tag="hhi")
                hlo = iop.tile([PC, MO, tn], f16, tag="hlo")
                whi, wlo = w_sb[(li, "hi")], w_sb[(li, "lo")]
                for mo in range(MO):
                    pt = pb.tile([PC, tn], f32, tag="pbig")
                    n3 = 3 * KC
                    i = 0
                    for wt, rt in ((whi, rhs_hi), (wlo, rhs_hi), (whi, rhs_lo)):
                        for c in range(KC):
                            nc.tensor.matmul(
                                pt[:], wt[:, c, bass.ts(mo, PC)], rt[:, c, :],
                                start=(i == 0), stop=(i == n3 - 1))
                            i += 1
                    nc.scalar.activation(
                        h32[:, mo, :], pt[:],
                        mybir.ActivationFunctionType.Relu,
                        bias=b_sb[li][:, mo:mo + 1])
                    nc.scalar.copy(hhi[:, mo, :], h32[:, mo, :])
                    nc.vector.tensor_tensor(
                        hlo[:, mo, :], h32[:, mo, :], hhi[:, mo, :],
                        mybir.AluOpType.subtract)
                    if mo == 1:
                        yield (h32, hhi, hlo)
                yield (h32, hhi, hlo)

            def produce(t):
                """Layers + experts + router logits for tile t (PE-dense,
                shallow cross-engine chains). Generator: yields at PE-dense
                stage boundaries so select() stages of the previous tile can
                be interleaved; final value is the state for select()."""
                tsl = slice(t * tn, (t + 1) * tn)
                if t == 0:
                    xhi, xlo = xhi0, xlo0   # prefetched before the weights
                else:
                    xhi = xp.tile([PC, KC, tn], f16, tag="xhi")
                    xlo = xp.tile([PC, KC, tn], f16, tag="xlo")
                    nc.sync.dma_start(xhi[:], xhi_r[:, :, tsl])
                    nc.sync.dma_start(xlo[:], xlo_r[:, :, tsl])

                g0 = split_layer(xhi, xlo, 0, relu=True)
                next(g0)
                _, h0hi, h0lo = next(g0)
                yield None
                g1 = split_layer(h0hi, h0lo, 1, relu=True)
                next(g1)
                yield None
                h1_32, mhi, mlo = next(g1)
                if stage < 2:
                    yield None
                    return

                # ---- experts: mu = Wgm m + bgm (single fp16) ----
                mu32 = sp.tile([PC, MUC, tn], f32, tag="mu32")
                for mo in range(MUC):
                    pt = pb.tile([PC, tn], f32, tag="pbig")
                    for c in range(KC):
                        nc.tensor.matmul(
                            pt[:], wgm_sb[:, c, bass.ts(mo, PC)], mhi[:, c, :],
                            start=(c == 0), stop=(c == KC - 1))
                    nc.vector.tensor_scalar_add(
                        mu32[:, mo, :], pt[:], bgm_sb[:, mo:mo + 1])
                if stage < 3:
                    yield (mu32, None)
                    return

                # ---- router logits (split fp16); the four kc chunks run
                # concurrently in the PE array's 32-column strips ----
                lg_ps = ps.tile([PC, tn], f32, tag="psm")
                for ti, (wt, rt) in enumerate(
                        ((wchi_sb, mhi), (wclo_sb, mhi), (wchi_sb, mlo))):
                    for c in range(KC):
                        nc.tensor.matmul(
                            lg_ps[32 * c:32 * c + K, :], wt[:, c, :],
                            rt[:, c, :], start=(ti == 0), stop=(ti == 2),
                            tile_position=(0, 32 * c),
                            skip_group_check=True)
                # gather the four 16-row partial strips onto partitions
                # 0-15 via DMA (cross-partition moves are DMA-only here),
                # then reduce with same-partition DVE adds
                lg4s = sp.tile([PC, tn], f32, tag="lg4s")
                for c in range(KC):
                    nc.vector.tensor_copy(lg4s[32 * c:32 * c + K, :],
                                          lg_ps[32 * c:32 * c + K, :])
                lg4 = sp.tile([K, KC, tn], f32, tag="lg4")
                for c in range(KC):
                    nc.sync.dma_start(lg4[:, c, :],
                                      lg4s[32 * c:32 * c + K, :])
                lga = sp.tile([K, tn], f32, tag="lga")
                nc.vector.tensor_tensor(lga[:], lg4[:, 0, :], lg4[:, 1, :],
                                        mybir.AluOpType.add)
                lgb = sp.tile([K, tn], f32, tag="lgb")
                nc.vector.tensor_tensor(lgb[:], lg4[:, 2, :], lg4[:, 3, :],
                                        mybir.AluOpType.add)
                lg = sp.tile([K, tn], f32, tag="lg")
                nc.vector.scalar_tensor_tensor(
                    lg[:], lga[:], bc_sb[:, 0:1], lgb[:],
                    op0=mybir.AluOpType.add, op1=mybir.AluOpType.add)
                yield (mu32, lg)

            def select(t, state):
                """Generator: yields between PE-visit stages."""
                if state is None or stage < 3:
                    return
                mu32, lg = state
                tsl = slice(t * tn, (t + 1) * tn)
                # ---- argmax: transpose logits to batch-major (exact fp32
                # identity matmul), rowwise max + is_equal, transpose the
                # 0/1 one-hot back (exact fp16 identity matmul) ----
                nch = tn // PC
                lgt_ps = ps.tile([PC, nch, K], f32, tag="psm")
                for c in range(nch):
                    nc.tensor.matmul(lgt_ps[:, c, :], lg[:, bass.ts(c, PC)],
                                     i16f_sb[:], start=(c == 0),
                                     stop=(c == nch - 1))
                yield
                # per-row argmax (first-match exact): min over e of
                # (e + BIG - BIG*[lg==max]); rebuild a guaranteed single-hot
                # mask from the index. All values are exact small ints.
                ohf_bm = sp.tile([PC, nch, K], f16, tag="ohbm")
                y_f = sp.tile([PC, nch], f32, tag="yf")
                for c in range(nch):
                    lmax = sp.tile([PC, 1], f32, tag="lmax")
                    nc.vector.tensor_reduce(
                        lmax[:], lgt_ps[:, c, :], axis=mybir.AxisListType.X,
                        op=mybir.AluOpType.max)
                    oh_c = sp.tile([PC, K], f16, tag="ohc")
                    nc.vector.tensor_scalar(
                        oh_c[:], lgt_ps[:, c, :], lmax[:, 0:1], None,
                        op0=mybir.AluOpType.is_equal)
                    val = sp.tile([PC, K], f32, tag="val")
                    nc.vector.scalar_tensor_tensor(
                        val[:], oh_c[:], -16.0, idxb_sb[:],
                        op0=mybir.AluOpType.mult, op1=mybir.AluOpType.add)
                    nc.vector.tensor_reduce(
                        y_f[:, c:c + 1], val[:], axis=mybir.AxisListType.X,
                        op=mybir.AluOpType.min)
                    nc.vector.tensor_scalar(
                        ohf_bm[:, c, :], idxc_sb[:], y_f[:, c:c + 1], None,
                        op0=mybir.AluOpType.is_equal)
                y_sb = sp.tile([PC, nch], i32, tag="ysb")
                nc.vector.tensor_copy(y_sb[:], y_f[:])
                if stage >= 4 and stage != 41:
                    nc.sync.dma_start(
                        y_d[tsl].rearrange("(c p) -> p c", p=PC), y_sb[:])
                yield
                ohf_ps = ps.tile([K, tn], f32, tag="psm")
                for c in range(nch):
                    nc.tensor.matmul(ohf_ps[:, bass.ts(c, PC)],
                                     ohf_bm[:, c, :], i128_sb[:],
                                     start=(c == 0), stop=(c == nch - 1))
                ohf = sp.tile([K, tn], f16, tag="ohf")
                nc.vector.tensor_copy(ohf[:], ohf_ps[:])
                yield
                if stage < 5:
                    return

                # ---- select expert mean: z = R^T (mu * (E^T ohf)) ----
                z_ps = ps.tile([K, tn], f32, tag="psm")
                for mo, e_sb in ((0, e0_sb), (1, e1_sb)):
                    msk_ps = pb.tile([PC, tn], f32, tag="pbig")
                    nc.tensor.matmul(msk_ps[:], e_sb[:], ohf[:],
                                     start=True, stop=True)
                    s_sb = sp.tile([PC, tn], f16, tag=f"s{mo}")
                    nc.vector.tensor_tensor(s_sb[:], mu32[:, mo, :], msk_ps[:],
                                            mybir.AluOpType.mult)
                    nc.tensor.matmul(z_ps[:], r_sb[:], s_sb[:],
                                     start=(mo == 0), stop=(mo == 1))
                z_sb = sp.tile([K, tn], f16, tag="zsb")
                nc.vector.tensor_copy(z_sb[:], z_ps[:])
                # transpose to batch-major [128, 16] chunks and store
                zb_ps = ps.tile([PC, nch, K], f32, tag="psm")
                for c in range(nch):
                    nc.tensor.matmul(zb_ps[:, c, :], z_sb[:, bass.ts(c, PC)],
                                     i16_sb[:], start=(c == 0),
                                     stop=(c == nch - 1))
                zb_sb = sp.tile([PC, nch, K], f32, tag="zbsb")
                nc.vector.tensor_copy(zb_sb[:], zb_ps[:])
                nc.sync.dma_start(
                    z_d[tsl, :].rearrange("(c p) e -> p c e", p=PC),
                    zb_sb[:])

            # Software pipeline: tile t-1's select stages (short PE visits
            # separated by DVE hops on tiny tensors) are interleaved between
            # tile t's PE-dense layer blocks, so the PE's in-order queue
            # always has dense work while each select stage's DVE inputs
            # complete.
            def drain(g):
                if g is not None:
                    for _ in g:
                        pass

            sels = {}
            for t in range(nt):
                pg = produce(t)
                next(pg)                      # L0
                if sels.get(t - 2) is not None:
                    drain(sels.pop(t - 2))    # stage C of t-2 (ohf ready)
                if sels.get(t - 1) is not None:
                    next(sels[t - 1], None)   # stage A of t-1 (argmax tr.)
                next(pg)                      # L1 first half
                if sels.get(t - 1) is not None:
                    next(sels[t - 1], None)   # stage A2 of t-1 (y / one-hot)
                state = next(pg)              # L1 rest + experts + router
                if sels.get(t - 1) is not None:
                    next(sels[t - 1], None)   # stage B of t-1 (ohf to fm)
                sels[t] = select(t, state)
            for t in sorted(sels):
                drain(sels[t])

    nc.compile()
    return nc


def _host_prep(x, W0, b0, W1, b1, W2, b2, Wc, bc, Wg, bg, bl=BL):
    """Build per-core input maps (numpy only)."""
    f16, f32 = np.float16, np.float32

    def split(a):
        hi = a.astype(f16)
        lo = (a.astype(f32) - hi.astype(f32)).astype(f16)
        return hi, lo

    xT = np.ascontiguousarray(np.asarray(x, f32).T)          # [D, B]
    xhi, xlo = split(xT)

    shared = {}
    for li, W in enumerate((W0, W1)):
        wT = np.ascontiguousarray(np.asarray(W, f32).T)       # [in, out]
        hi, lo = split(wT)
        shared[f"w{li}hi"] = hi
        shared[f"w{li}lo"] = lo
    # Fuse the identity-output layer W2 into the router and the experts
    # (host-side, fp64): logits = (Wc W2) h1 + (Wc b2 + bc),
    # mu = (Wgm W2) h1 + (Wgm b2 + bgm).
    f64 = np.float64
    W2_, b2_ = np.asarray(W2, f64), np.asarray(b2, f64)
    wfc = np.asarray(Wc, f64) @ W2_                            # [K, D]
    bfc = np.asarray(Wc, f64) @ b2_ + np.asarray(bc, f64)
    wgm2 = np.asarray(Wg, f64)[:, :LAT, :].reshape(K * LAT, D)
    wfg = wgm2 @ W2_                                           # [MU, D]
    bfg = wgm2 @ b2_ + np.asarray(bg, f64)[:, :LAT].reshape(-1)
    shared["wchi"], shared["wclo"] = split(
        np.ascontiguousarray(wfc.T).astype(f32))
    shared["wgm"] = np.ascontiguousarray(wfg.T).astype(f16)
    shared["b0"] = np.asarray(b0, f32)
    shared["b1"] = np.asarray(b1, f32)
    shared["bgm"] = bfg.astype(f32)
    shared["bc"] = bfc.astype(f32)

    kk = np.arange(K)
    pp = np.arange(PC)
    e0 = (pp[None, :] // LAT == kk[:, None]).astype(f16)
    e0[8:, :] = 0
    e1 = ((pp[None, :] // LAT + 8) == kk[:, None]).astype(f16)
    shared["e0"] = e0
    shared["e1"] = e1
    shared["r"] = (pp[:, None] % LAT == np.arange(LAT)[None, :]).astype(f16)
    shared["i16"] = np.eye(K, dtype=f16)
    shared["i16f"] = np.eye(K, dtype=f32)
    shared["i128"] = np.eye(PC, dtype=f16)
    shared["idxb"] = np.broadcast_to(kk.astype(f32) + 16.0, (PC, K)).copy()
    shared["idxc"] = np.broadcast_to(kk.astype(f32), (PC, K)).copy()

    ncores = xhi.shape[1] // bl
    in_maps = []
    for c in range(ncores):
        m = dict(shared)
        m["xhi"] = np.ascontiguousarray(xhi[:, c * bl:(c + 1) * bl])
        m["xlo"] = np.ascontiguousarray(xlo[:, c * bl:(c + 1) * bl])
        in_maps.append(m)
    return in_maps


def kernel(x, W0, b0, W1, b1, W2, b2, Wc, bc, Wg, bg, trace=False):
    from concourse.bass_utils import run_bass_kernel_spmd

    if "nc" not in _CACHE:
        _CACHE["nc"] = build_program()
    nc = _CACHE["nc"]

    in_maps = _host_prep(x, W0, b0, W1, b1, W2, b2, Wc, bc, Wg, bg)
    res = run_bass_kernel_spmd(nc, in_maps, core_ids=list(range(NCORES)),
                               trace=trace)
    z = np.concatenate([res.results[c]["z"] for c in range(NCORES)], axis=0)
    y = np.concatenate([res.results[c]["y"] for c in range(NCORES)], axis=0)
    if trace:
        kernel.last_results = res
    return z, y.astype(np.int32)


kernel.last_results = None


# revision 54
# speedup vs baseline: 1.0100x; 1.0100x over previous
"""Trainium2 Bass kernel for nn_EncoderMixtureModelTrajectory.

Model: 3-layer ReLU MLP (512->512->512->512) -> softmax router over 16
classes (argmax only is needed) -> per-class gaussian expert means
(mu = first 16 of 32 outputs), gather selected class's mu per sample.

Strategy: pure data-parallel over 8 NeuronCores (batch 65536 -> 8192/core).
Feature-major activations [D, N] on-chip; host pre-transposes x and weights.
Precision: fp16 hi/lo split matmuls (3 fp16 matmuls/pair ~ fp32 accuracy)
for the m-chain + router (the argmax decision is precision-critical);
single fp16 for the expert layer and selection machinery.
Router argmax + expert selection are done with small matmuls on the PE
(max-tree on DVE, one-hot via is_equal, first-match via triangular matmul,
select via expand/contract 0-1 matrices, batch-major transpose via identity
matmul).
"""

import os
import sys

if "/opt/trn_rl_repo" not in sys.path:
    sys.path.insert(0, "/opt/trn_rl_repo")
# Recover automatically if a previous run left a NeuronCore wedged.
os.environ.setdefault("NEURON_RT_RESET_CORES", "1")

import numpy as np

# Problem constants (hardcoded per task contract)
B = 65536
D = 512
LAT = 16
K = 16
NCORES = 8
BL = B // NCORES          # 8192 samples per core
TN = 512                  # batch-tile (free dim per matmul)
PC = 128                  # partitions
KC = D // PC              # 4 contraction chunks
MO = D // PC              # 4 output chunks for D-wide layers
MU = K * LAT              # 256 expert-mean rows
MUC = MU // PC            # 2 chunks

_CACHE = {}


def _dt():
    import concourse.mybir as mybir
    return mybir.dt


def build_program(bl=BL, tn=TN, stage=5):
    """Build the Bass/Tile program for one core (SPMD across 8).

    stage: debug bisection level (5 = full kernel)."""
    import concourse.bass as bass
    import concourse.bacc as bacc
    import concourse.mybir as mybir
    import concourse.tile as tile

    dt = mybir.dt
    f16, f32, i32 = dt.float16, dt.float32, dt.int32
    nt = bl // tn

    nc = bacc.Bacc("TRN2", target_bir_lowering=False, debug=False,
                   num_devices=NCORES)

    def din(name, shape, dtype):
        return nc.dram_tensor(name, shape, dtype, kind="ExternalInput").ap()

    def dout(name, shape, dtype):
        return nc.dram_tensor(name, shape, dtype, kind="ExternalOutput").ap()

    xhi_d = din("xhi", [D, bl], f16)
    xlo_d = din("xlo", [D, bl], f16)
    w_d = {}
    for li in range(2):
        for part in ("hi", "lo"):
            w_d[(li, part)] = din(f"w{li}{part}", [D, D], f16)
    wchi_d = din("wchi", [D, K], f16)
    wclo_d = din("wclo", [D, K], f16)
    wgm_d = din("wgm", [D, MU], f16)
    b_d = [din(f"b{li}", [D], f32) for li in range(2)]
    bgm_d = din("bgm", [MU], f32)
    bc_d = din("bc", [K], f32)
    e0_d = din("e0", [K, PC], f16)
    e1_d = din("e1", [K, PC], f16)
    r_d = din("r", [PC, K], f16)
    i16_d = din("i16", [K, K], f16)
    i16f_d = din("i16f", [K, K], f32)
    i128_d = din("i128", [PC, PC], f16)
    idxb_d = din("idxb", [PC, K], f32)
    idxc_d = din("idxc", [PC, K], f32)

    z_d = dout("z", [bl, LAT], f32)
    y_d = dout("y", [bl], i32)

    with tile.TileContext(nc) as tc:
        with (
            tc.tile_pool(name="wpool", bufs=1) as wp,
            tc.tile_pool(name="xpool", bufs=3) as xp,
            tc.tile_pool(name="hpool", bufs=2) as hp,
            tc.tile_pool(name="iopool", bufs=3) as iop,
            tc.tile_pool(name="spool", bufs=3) as sp,
            tc.tile_pool(name="pbig", bufs=4, space="PSUM") as pb,
            tc.tile_pool(name="psmall", bufs=4, space="PSUM") as ps,
        ):
            # ---- resident weights/constants ----
            # w0 (needed first) goes on the sync queue; everything else on
            # the gpsimd queue so tile-0's x/w0 DMAs aren't queued behind it.
            w_sb = {}
            for key, d in w_d.items():
                t = wp.tile([PC, KC, D], f16, tag=f"w{key[0]}{key[1]}")
                q = nc.sync if key[0] == 0 else nc.gpsimd
                q.dma_start(t[:], d.rearrange("(c p) o -> p c o", p=PC))
                w_sb[key] = t
            wchi_sb = wp.tile([PC, KC, K], f16, tag="wchi")
            nc.gpsimd.dma_start(wchi_sb[:], wchi_d.rearrange("(c p) o -> p c o", p=PC))
            wclo_sb = wp.tile([PC, KC, K], f16, tag="wclo")
            nc.gpsimd.dma_start(wclo_sb[:], wclo_d.rearrange("(c p) o -> p c o", p=PC))
            wgm_sb = wp.tile([PC, KC, MU], f16, tag="wgm")
            nc.gpsimd.dma_start(wgm_sb[:], wgm_d.rearrange("(c p) o -> p c o", p=PC))

            b_sb = []
            for li in range(2):
                t = wp.tile([PC, MO], f32, tag=f"b{li}")
                nc.gpsimd.dma_start(t[:], b_d[li].rearrange("(c p) -> p c", p=PC))
                b_sb.append(t)
            bgm_sb = wp.tile([PC, MUC], f32, tag="bgm")
            nc.gpsimd.dma_start(bgm_sb[:], bgm_d.rearrange("(c p) -> p c", p=PC))
            bc_sb = wp.tile([K, 1], f32, tag="bc")
            nc.gpsimd.dma_start(bc_sb[:], bc_d.rearrange("(k o) -> k o", o=1))
            e0_sb = wp.tile([K, PC], f16, tag="e0")
            nc.gpsimd.dma_start(e0_sb[:], e0_d[:])
            e1_sb = wp.tile([K, PC], f16, tag="e1")
            nc.gpsimd.dma_start(e1_sb[:], e1_d[:])
            r_sb = wp.tile([PC, K], f16, tag="r")
            nc.gpsimd.dma_start(r_sb[:], r_d[:])
            i16_sb = wp.tile([K, K], f16, tag="i16")
            nc.gpsimd.dma_start(i16_sb[:], i16_d[:])
            idxb_sb = wp.tile([PC, K], f32, tag="idxb")
            nc.gpsimd.dma_start(idxb_sb[:], idxb_d[:])
            idxc_sb = wp.tile([PC, K], f32, tag="idxc")
            nc.gpsimd.dma_start(idxc_sb[:], idxc_d[:])
            i16f_sb = wp.tile([K, K], f32, tag="i16f")
            nc.gpsimd.dma_start(i16f_sb[:], i16f_d[:])
            i128_sb = wp.tile([PC, PC], f16, tag="i128")
            nc.gpsimd.dma_start(i128_sb[:], i128_d[:])

            xhi_r = xhi_d.rearrange("(c p) n -> p c n", p=PC)
            xlo_r = xlo_d.rearrange("(c p) n -> p c n", p=PC)

            def split_layer(rhs_hi, rhs_lo, li, relu):
                """rhs_{hi,lo}: [PC, KC, tn] f16 -> returns (hi, lo) fp16
                tiles [PC, MO, tn] of relu(W x + b).

                hhi comes straight off the ACT engine (fp16 write rounds);
                the fp32 relu and the residual are computed on DVE in
                parallel, so the next layer's hi-term matmuls (issued
                first) only wait one ACT hop."""
                assert relu
                h32 = hp.tile([PC, MO, tn], f32, tag="h32")
                hhi = iop.tile([PC, MO, tn], f16, tag="hhi")
                hlo = iop.tile([PC, MO, tn], f16, tag="hlo")
                whi, wlo = w_sb[(li, "hi")], w_sb[(li, "lo")]
                for mo in range(MO):
                    pt = pb.tile([PC, tn], f32, tag="pbig")
                    n3 = 3 * KC
                    i = 0
                    for wt, rt in ((whi, rhs_hi), (wlo, rhs_hi), (whi, rhs_lo)):
                        for c in range(KC):
                            nc.tensor.matmul(
                                pt[:], wt[:, c, bass.ts(mo, PC)], rt[:, c, :],
                                start=(i == 0), stop=(i == n3 - 1))
                            i += 1
                    nc.scalar.activation(
                        h32[:, mo, :], pt[:],
                        mybir.ActivationFunctionType.Relu,
                        bias=b_sb[li][:, mo:mo + 1])
                    nc.scalar.copy(hhi[:, mo, :], h32[:, mo, :])
                    nc.vector.tensor_tensor(
                        hlo[:, mo, :], h32[:, mo, :], hhi[:, mo, :],
                        mybir.AluOpType.subtract)
                    if mo == 1:
                        yield (h32, hhi, hlo)
                yield (h32, hhi, hlo)

            def produce(t):
                """Layers + experts + router logits for tile t (PE-dense,
                shallow cross-engine chains). Generator: yields at PE-dense
                stage boundaries so select() stages of the previous tile can
                be interleaved; final value is the state for select()."""
                tsl = slice(t * tn, (t + 1) * tn)
                xhi = xp.tile([PC, KC, tn], f16, tag="xhi")
                xlo = xp.tile([PC, KC, tn], f16, tag="xlo")
                if t == 0:
                    # split the cold-start load across queues
                    nc.sync.dma_start(xhi[:, 0:2, :], xhi_r[:, 0:2, tsl])
                    nc.scalar.dma_start(xhi[:, 2:4, :], xhi_r[:, 2:4, tsl])
                    nc.scalar.dma_start(xlo[:, 0:2, :], xlo_r[:, 0:2, tsl])
                    nc.sync.dma_start(xlo[:, 2:4, :], xlo_r[:, 2:4, tsl])
                else:
                    nc.sync.dma_start(xhi[:], xhi_r[:, :, tsl])
                    nc.sync.dma_start(xlo[:], xlo_r[:, :, tsl])

                g0 = split_layer(xhi, xlo, 0, relu=True)
                next(g0)
                _, h0hi, h0lo = next(g0)
                yield None
                g1 = split_layer(h0hi, h0lo, 1, relu=True)
                next(g1)
                yield None
                h1_32, mhi, mlo = next(g1)
                if stage < 2:
                    yield None
                    return

                # ---- experts: mu = Wgm m + bgm (single fp16) ----
                mu32 = sp.tile([PC, MUC, tn], f32, tag="mu32")
                for mo in range(MUC):
                    pt = pb.tile([PC, tn], f32, tag="pbig")
                    for c in range(KC):
                        nc.tensor.matmul(
                            pt[:], wgm_sb[:, c, bass.ts(mo, PC)], mhi[:, c, :],
                            start=(c == 0), stop=(c == KC - 1))
                    nc.vector.tensor_scalar_add(
                        mu32[:, mo, :], pt[:], bgm_sb[:, mo:mo + 1])
                if stage < 3:
                    yield (mu32, None)
                    return

                # ---- router logits (split fp16); the four kc chunks run
                # concurrently in the PE array's 32-column strips ----
                lg_ps = ps.tile([PC, tn], f32, tag="psm")
                for ti, (wt, rt) in enumerate(
                        ((wchi_sb, mhi), (wclo_sb, mhi), (wchi_sb, mlo))):
                    for c in range(KC):
                        nc.tensor.matmul(
                            lg_ps[32 * c:32 * c + K, :], wt[:, c, :],
                            rt[:, c, :], start=(ti == 0), stop=(ti == 2),
                            tile_position=(0, 32 * c),
                            skip_group_check=True)
                # gather the four 16-row partial strips onto partitions
                # 0-15 via DMA (cross-partition moves are DMA-only here),
                # then reduce with same-partition DVE adds
                lg4s = sp.tile([PC, tn], f32, tag="lg4s")
                for c in range(KC):
                    nc.vector.tensor_copy(lg4s[32 * c:32 * c + K, :],
                                          lg_ps[32 * c:32 * c + K, :])
                lg4 = sp.tile([K, KC, tn], f32, tag="lg4")
                for c in range(KC):
                    nc.sync.dma_start(lg4[:, c, :],
                                      lg4s[32 * c:32 * c + K, :])
                lga = sp.tile([K, tn], f32, tag="lga")
                nc.vector.tensor_tensor(lga[:], lg4[:, 0, :], lg4[:, 1, :],
                                        mybir.AluOpType.add)
                lgb = sp.tile([K, tn], f32, tag="lgb")
                nc.vector.tensor_tensor(lgb[:], lg4[:, 2, :], lg4[:, 3, :],
                                        mybir.AluOpType.add)
                lg = sp.tile([K, tn], f32, tag="lg")
                nc.vector.scalar_tensor_tensor(
                    lg[:], lga[:], bc_sb[:, 0:1], lgb[:],
                    op0=mybir.AluOpType.add, op1=mybir.AluOpType.add)
                yield (mu32, lg)

            def select(t, state):
                """Generator: yields between PE-visit stages."""
                if state is None or stage < 3:
                    return
                mu32, lg = state
                tsl = slice(t * tn, (t + 1) * tn)
                # ---- argmax: transpose logits to batch-major (exact fp32
                # identity matmul), rowwise max + is_equal, transpose the
                # 0/1 one-hot back (exact fp16 identity matmul) ----
                nch = tn // PC
                lgt_ps = ps.tile([PC, nch, K], f32, tag="psm")
                for c in range(nch):
                    nc.tensor.matmul(lgt_ps[:, c, :], lg[:, bass.ts(c, PC)],
                                     i16f_sb[:], start=(c == 0),
                                     stop=(c == nch - 1))
                yield
                # per-row argmax (first-match exact): min over e of
                # (e + BIG - BIG*[lg==max]); rebuild a guaranteed single-hot
                # mask from the index. All values are exact small ints.
                ohf_bm = sp.tile([PC, nch, K], f16, tag="ohbm")
                y_f = sp.tile([PC, nch], f32, tag="yf")
                for c in range(nch):
                    lmax = sp.tile([PC, 1], f32, tag="lmax")
                    nc.vector.tensor_reduce(
                        lmax[:], lgt_ps[:, c, :], axis=mybir.AxisListType.X,
                        op=mybir.AluOpType.max)
                    oh_c = sp.tile([PC, K], f16, tag="ohc")
                    nc.vector.tensor_scalar(
                        oh_c[:], lgt_ps[:, c, :], lmax[:, 0:1], None,
                        op0=mybir.AluOpType.is_equal)
                    val = sp.tile([PC, K], f32, tag="val")
                    nc.vector.scalar_tensor_tensor(
                        val[:], oh_c[:], -16.0, idxb_sb[:],
                        op0=mybir.AluOpType.mult, op1=mybir.AluOpType.add)
                    nc.vector.tensor_reduce(
                        y_f[:, c:c + 1], val[:], axis=mybir.AxisListType.X,
                        op=mybir.AluOpType.min)
                    nc.vector.tensor_scalar(
                        ohf_bm[:, c, :], idxc_sb[:], y_f[:, c:c + 1], None,
                        op0=mybir.AluOpType.is_equal)
                y_sb = sp.tile([PC, nch], i32, tag="ysb")
                nc.vector.tensor_copy(y_sb[:], y_f[:])
                if stage >= 4 and stage != 41:
                    nc.sync.dma_start(
                        y_d[tsl].rearrange("(c p) -> p c", p=PC), y_sb[:])
                yield
                ohf_ps = ps.tile([K, tn], f32, tag="psm")
                for c in range(nch):
                    nc.tensor.matmul(ohf_ps[:, bass.ts(c, PC)],
                                     ohf_bm[:, c, :], i128_sb[:],
                                     start=(c == 0), stop=(c == nch - 1))
                ohf = sp.tile([K, tn], f16, tag="ohf")
                nc.vector.tensor_copy(ohf[:], ohf_ps[:])
                yield
                if stage < 5:
                    return

                # ---- select expert mean: z = R^T (mu * (E^T ohf)) ----
                z_ps = ps.tile([K, tn], f32, tag="psm")
                for mo, e_sb in ((0, e0_sb), (1, e1_sb)):
                    msk_ps = pb.tile([PC, tn], f32, tag="pbig")
                    nc.tensor.matmul(msk_ps[:], e_sb[:], ohf[:],
                                     start=True, stop=True)
                    s_sb = sp.tile([PC, tn], f16, tag=f"s{mo}")
                    nc.vector.tensor_tensor(s_sb[:], mu32[:, mo, :], msk_ps[:],
                                            mybir.AluOpType.mult)
                    nc.tensor.matmul(z_ps[:], r_sb[:], s_sb[:],
                                     start=(mo == 0), stop=(mo == 1))
                z_sb = sp.tile([K, tn], f16, tag="zsb")
                nc.vector.tensor_copy(z_sb[:], z_ps[:])
                # transpose to batch-major [128, 16] chunks and store
                zb_ps = ps.tile([PC, nch, K], f32, tag="psm")
                for c in range(nch):
                    nc.tensor.matmul(zb_ps[:, c, :], z_sb[:, bass.ts(c, PC)],
                                     i16_sb[:], start=(c == 0),
                                     stop=(c == nch - 1))
                zb_sb = sp.tile([PC, nch, K], f32, tag="zbsb")
                nc.vector.tensor_copy(zb_sb[:], zb_ps[:])
                nc.sync.dma_start(
                    z_d[tsl, :].rearrange("(c p) e -> p c e", p=PC),
                    zb_sb[:])

            # Software pipeline: tile t-1's select stages (short PE visits
            # separated by DVE hops on tiny tensors) are interleaved between
            # tile t's PE-dense layer blocks, so the PE's in-order queue
            # always has dense work while each select stage's DVE inputs
            # complete.
            def drain(g):
                if g is not None:
                    for _ in g:
                        pass

            sels = {}
            for t in range(nt):
                pg = produce(t)
                next(pg)                      # L0
                if sels.get(t - 2) is not None:
                    drain(sels.pop(t - 2))    # stage C of t-2 (ohf ready)
                if sels.get(t - 1) is not None:
                    next(sels[t - 1], None)   # stage A of t-1 (argmax tr.)
                next(pg)                      # L1 first half
                if sels.get(t - 1) is not None:
                    next(sels[t - 1], None)   # stage A2 of t-1 (y / one-hot)
                state = next(pg)              # L1 rest + experts + router
                if sels.get(t - 1) is not None:
                    next(sels[t - 1], None)   # stage B of t-1 (ohf to fm)
                sels[t] = select(t, state)
            for t in sorted(sels):
                drain(sels[t])

    nc.compile()
    return nc


def _host_prep(x, W0, b0, W1, b1, W2, b2, Wc, bc, Wg, bg, bl=BL):
    """Build per-core input maps (numpy only)."""
    f16, f32 = np.float16, np.float32

    def split(a):
        hi = a.astype(f16)
        lo = (a.astype(f32) - hi.astype(f32)).astype(f16)
        return hi, lo

    xT = np.ascontiguousarray(np.asarray(x, f32).T)          # [D, B]
    xhi, xlo = split(xT)

    shared = {}
    for li, W in enumerate((W0, W1)):
        wT = np.ascontiguousarray(np.asarray(W, f32).T)       # [in, out]
        hi, lo = split(wT)
        shared[f"w{li}hi"] = hi
        shared[f"w{li}lo"] = lo
    # Fuse the identity-output layer W2 into the router and the experts
    # (host-side, fp64): logits = (Wc W2) h1 + (Wc b2 + bc),
    # mu = (Wgm W2) h1 + (Wgm b2 + bgm).
    f64 = np.float64
    W2_, b2_ = np.asarray(W2, f64), np.asarray(b2, f64)
    wfc = np.asarray(Wc, f64) @ W2_                            # [K, D]
    bfc = np.asarray(Wc, f64) @ b2_ + np.asarray(bc, f64)
    wgm2 = np.asarray(Wg, f64)[:, :LAT, :].reshape(K * LAT, D)
    wfg = wgm2 @ W2_                                           # [MU, D]
    bfg = wgm2 @ b2_ + np.asarray(bg, f64)[:, :LAT].reshape(-1)
    shared["wchi"], shared["wclo"] = split(
        np.ascontiguousarray(wfc.T).astype(f32))
    shared["wgm"] = np.ascontiguousarray(wfg.T).astype(f16)
    shared["b0"] = np.asarray(b0, f32)
    shared["b1"] = np.asarray(b1, f32)
    shared["bgm"] = bfg.astype(f32)
    shared["bc"] = bfc.astype(f32)

    kk = np.arange(K)
    pp = np.arange(PC)
    e0 = (pp[None, :] // LAT == kk[:, None]).astype(f16)
    e0[8:, :] = 0
    e1 = ((pp[None, :] // LAT + 8) == kk[:, None]).astype(f16)
    shared["e0"] = e0
    shared["e1"] = e1
    shared["r"] = (pp[:, None] % LAT == np.arange(LAT)[None, :]).astype(f16)
    shared["i16"] = np.eye(K, dtype=f16)
    shared["i16f"] = np.eye(K, dtype=f32)
    shared["i128"] = np.eye(PC, dtype=f16)
    shared["idxb"] = np.broadcast_to(kk.astype(f32) + 16.0, (PC, K)).copy()
    shared["idxc"] = np.broadcast_to(kk.astype(f32), (PC, K)).copy()

    ncores = xhi.shape[1] // bl
    in_maps = []
    for c in range(ncores):
        m = dict(shared)
        m["xhi"] = np.ascontiguousarray(xhi[:, c * bl:(c + 1) * bl])
        m["xlo"] = np.ascontiguousarray(xlo[:, c * bl:(c + 1) * bl])
        in_maps.append(m)
    return in_maps


def kernel(x, W0, b0, W1, b1, W2, b2, Wc, bc, Wg, bg, trace=False):
    from concourse.bass_utils import run_bass_kernel_spmd

    if "nc" not in _CACHE:
        _CACHE["nc"] = build_program()
    nc = _CACHE["nc"]

    in_maps = _host_prep(x, W0, b0, W1, b1, W2, b2, Wc, bc, Wg, bg)
    res = run_bass_kernel_spmd(nc, in_maps, core_ids=list(range(NCORES)),
                               trace=trace)
    z = np.concatenate([res.results[c]["z"] for c in range(NCORES)], axis=0)
    y = np.concatenate([res.results[c]["y"] for c in range(NCORES)], axis=0)
    if trace:
        kernel.last_results = res
    return z, y.astype(np.int32)


kernel.last_results = None


# revision 55
# speedup vs baseline: 1.0258x; 1.0156x over previous
"""Trainium2 Bass kernel for nn_EncoderMixtureModelTrajectory.

Model: 3-layer ReLU MLP (512->512->512->512) -> softmax router over 16
classes (argmax only is needed) -> per-class gaussian expert means
(mu = first 16 of 32 outputs), gather selected class's mu per sample.

Strategy: pure data-parallel over 8 NeuronCores (batch 65536 -> 8192/core).
Feature-major activations [D, N] on-chip; host pre-transposes x and weights.
Precision: fp16 hi/lo split matmuls (3 fp16 matmuls/pair ~ fp32 accuracy)
for the m-chain + router (the argmax decision is precision-critical);
single fp16 for the expert layer and selection machinery.
Router argmax + expert selection are done with small matmuls on the PE
(max-tree on DVE, one-hot via is_equal, first-match via triangular matmul,
select via expand/contract 0-1 matrices, batch-major transpose via identity
matmul).
"""

import os
import sys

if "/opt/trn_rl_repo" not in sys.path:
    sys.path.insert(0, "/opt/trn_rl_repo")
# Recover automatically if a previous run left a NeuronCore wedged.
os.environ.setdefault("NEURON_RT_RESET_CORES", "1")

import numpy as np

# Problem constants (hardcoded per task contract)
B = 65536
D = 512
LAT = 16
K = 16
NCORES = 8
BL = B // NCORES          # 8192 samples per core
TN = 512                  # batch-tile (free dim per matmul)
PC = 128                  # partitions
KC = D // PC              # 4 contraction chunks
MO = D // PC              # 4 output chunks for D-wide layers
MU = K * LAT              # 256 expert-mean rows
MUC = MU // PC            # 2 chunks

_CACHE = {}


def _dt():
    import concourse.mybir as mybir
    return mybir.dt


def build_program(bl=BL, tn=TN, stage=5):
    """Build the Bass/Tile program for one core (SPMD across 8).

    stage: debug bisection level (5 = full kernel)."""
    import concourse.bass as bass
    import concourse.bacc as bacc
    import concourse.mybir as mybir
    import concourse.tile as tile

    dt = mybir.dt
    f16, f32, i32 = dt.float16, dt.float32, dt.int32
    nt = bl // tn

    nc = bacc.Bacc("TRN2", target_bir_lowering=False, debug=False,
                   num_devices=NCORES)

    def din(name, shape, dtype):
        return nc.dram_tensor(name, shape, dtype, kind="ExternalInput").ap()

    def dout(name, shape, dtype):
        return nc.dram_tensor(name, shape, dtype, kind="ExternalOutput").ap()

    xhi_d = din("xhi", [D, bl], f16)
    xlo_d = din("xlo", [D, bl], f16)
    w_d = {}
    for li in range(2):
        for part in ("hi", "lo"):
            w_d[(li, part)] = din(f"w{li}{part}", [D, D], f16)
    wchi_d = din("wchi", [D, K], f16)
    wclo_d = din("wclo", [D, K], f16)
    wgm_d = din("wgm", [D, MU], f16)
    b_d = [din(f"b{li}", [D], f32) for li in range(2)]
    bgm_d = din("bgm", [MU], f32)
    bc_d = din("bc", [K], f32)
    e0_d = din("e0", [K, PC], f16)
    e1_d = din("e1", [K, PC], f16)
    r_d = din("r", [PC, K], f16)
    i16f_d = din("i16f", [K, K], f32)
    i128_d = din("i128", [PC, PC], f16)
    idxb_d = din("idxb", [PC, K], f32)
    idxc_d = din("idxc", [PC, K], f32)

    z_d = dout("z", [LAT, bl], f32)
    y_d = dout("y", [bl], i32)

    with tile.TileContext(nc) as tc:
        with (
            tc.tile_pool(name="wpool", bufs=1) as wp,
            tc.tile_pool(name="xpool", bufs=3) as xp,
            tc.tile_pool(name="hpool", bufs=2) as hp,
            tc.tile_pool(name="iopool", bufs=3) as iop,
            tc.tile_pool(name="spool", bufs=3) as sp,
            tc.tile_pool(name="pbig", bufs=4, space="PSUM") as pb,
            tc.tile_pool(name="psmall", bufs=4, space="PSUM") as ps,
        ):
            # ---- resident weights/constants ----
            # w0 (needed first) goes on the sync queue; everything else on
            # the gpsimd queue so tile-0's x/w0 DMAs aren't queued behind it.
            xhi_r = xhi_d.rearrange("(c p) n -> p c n", p=PC)
            xlo_r = xlo_d.rearrange("(c p) n -> p c n", p=PC)
            xhi0 = xp.tile([PC, KC, tn], f16, tag="xhi")
            xlo0 = xp.tile([PC, KC, tn], f16, tag="xlo")
            nc.gpsimd.dma_start(xhi0[:], xhi_r[:, :, 0:tn])
            nc.gpsimd.dma_start(xlo0[:], xlo_r[:, :, 0:tn])
            w_sb = {}
            for key, d in w_d.items():
                t = wp.tile([PC, KC, D], f16, tag=f"w{key[0]}{key[1]}")
                w_sb[key] = t
            # w0 lands mo-chunk-interleaved so L0's first group starts early
            for mo in range(MO):
                for part in ("hi", "lo"):
                    nc.sync.dma_start(
                        w_sb[(0, part)][:, :, bass.ts(mo, PC)],
                        w_d[(0, part)].rearrange(
                            "(c p) o -> p c o", p=PC)[:, :, bass.ts(mo, PC)])
            for part in ("hi", "lo"):
                nc.gpsimd.dma_start(
                    w_sb[(1, part)][:],
                    w_d[(1, part)].rearrange("(c p) o -> p c o", p=PC))
            wchi_sb = wp.tile([PC, KC, K], f16, tag="wchi")
            nc.gpsimd.dma_start(wchi_sb[:], wchi_d.rearrange("(c p) o -> p c o", p=PC))
            wclo_sb = wp.tile([PC, KC, K], f16, tag="wclo")
            nc.gpsimd.dma_start(wclo_sb[:], wclo_d.rearrange("(c p) o -> p c o", p=PC))
            wgm_sb = wp.tile([PC, KC, MU], f16, tag="wgm")
            nc.gpsimd.dma_start(wgm_sb[:], wgm_d.rearrange("(c p) o -> p c o", p=PC))

            b_sb = []
            for li in range(2):
                t = wp.tile([PC, MO], f32, tag=f"b{li}")
                nc.gpsimd.dma_start(t[:], b_d[li].rearrange("(c p) -> p c", p=PC))
                b_sb.append(t)
            bgm_sb = wp.tile([PC, MUC], f32, tag="bgm")
            nc.gpsimd.dma_start(bgm_sb[:], bgm_d.rearrange("(c p) -> p c", p=PC))
            bc_sb = wp.tile([K, 1], f32, tag="bc")
            nc.gpsimd.dma_start(bc_sb[:], bc_d.rearrange("(k o) -> k o", o=1))
            e0_sb = wp.tile([K, PC], f16, tag="e0")
            nc.gpsimd.dma_start(e0_sb[:], e0_d[:])
            e1_sb = wp.tile([K, PC], f16, tag="e1")
            nc.gpsimd.dma_start(e1_sb[:], e1_d[:])
            r_sb = wp.tile([PC, K], f16, tag="r")
            nc.gpsimd.dma_start(r_sb[:], r_d[:])
            idxb_sb = wp.tile([PC, K], f32, tag="idxb")
            nc.gpsimd.dma_start(idxb_sb[:], idxb_d[:])
            idxc_sb = wp.tile([PC, K], f32, tag="idxc")
            nc.gpsimd.dma_start(idxc_sb[:], idxc_d[:])
            i16f_sb = wp.tile([K, K], f32, tag="i16f")
            nc.gpsimd.dma_start(i16f_sb[:], i16f_d[:])
            i128_sb = wp.tile([PC, PC], f16, tag="i128")
            nc.gpsimd.dma_start(i128_sb[:], i128_d[:])


            def split_layer(rhs_hi, rhs_lo, li, relu):
                """rhs_{hi,lo}: [PC, KC, tn] f16 -> returns (hi, lo) fp16
                tiles [PC, MO, tn] of relu(W x + b).

                hhi comes straight off the ACT engine (fp16 write rounds);
                the fp32 relu and the residual are computed on DVE in
                parallel, so the next layer's hi-term matmuls (issued
                first) only wait one ACT hop."""
                assert relu
                h32 = hp.tile([PC, MO, tn], f32, tag="h32")
                hhi = iop.tile([PC, MO, tn], f16, tag="hhi")
                hlo = iop.tile([PC, MO, tn], f16, tag="hlo")
                whi, wlo = w_sb[(li, "hi")], w_sb[(li, "lo")]
                for mo in range(MO):
                    pt = pb.tile([PC, tn], f32, tag="pbig")
                    n3 = 3 * KC
                    i = 0
                    for wt, rt in ((whi, rhs_hi), (wlo, rhs_hi), (whi, rhs_lo)):
                        for c in range(KC):
                            nc.tensor.matmul(
                                pt[:], wt[:, c, bass.ts(mo, PC)], rt[:, c, :],
                                start=(i == 0), stop=(i == n3 - 1))
                            i += 1
                    nc.scalar.activation(
                        h32[:, mo, :], pt[:],
                        mybir.ActivationFunctionType.Relu,
                        bias=b_sb[li][:, mo:mo + 1])
                    nc.scalar.copy(hhi[:, mo, :], h32[:, mo, :])
                    nc.vector.tensor_tensor(
                        hlo[:, mo, :], h32[:, mo, :], hhi[:, mo, :],
                        mybir.AluOpType.subtract)
                    if mo == 1:
                        yield (h32, hhi, hlo)
                yield (h32, hhi, hlo)

            def produce(t):
                """Layers + experts + router logits for tile t (PE-dense,
                shallow cross-engine chains). Generator: yields at PE-dense
                stage boundaries so select() stages of the previous tile can
                be interleaved; final value is the state for select()."""
                tsl = slice(t * tn, (t + 1) * tn)
                if t == 0:
                    xhi, xlo = xhi0, xlo0   # prefetched before the weights
                else:
                    xhi = xp.tile([PC, KC, tn], f16, tag="xhi")
                    xlo = xp.tile([PC, KC, tn], f16, tag="xlo")
                    nc.sync.dma_start(xhi[:], xhi_r[:, :, tsl])
                    nc.sync.dma_start(xlo[:], xlo_r[:, :, tsl])

                g0 = split_layer(xhi, xlo, 0, relu=True)
                next(g0)
                _, h0hi, h0lo = next(g0)
                yield None
                g1 = split_layer(h0hi, h0lo, 1, relu=True)
                next(g1)
                yield None
                h1_32, mhi, mlo = next(g1)
                if stage < 2:
                    yield None
                    return

                # ---- experts: mu = Wgm m + bgm (single fp16) ----
                mu32 = sp.tile([PC, MUC, tn], f32, tag="mu32")
                for mo in range(MUC):
                    pt = pb.tile([PC, tn], f32, tag="pbig")
                    for c in range(KC):
                        nc.tensor.matmul(
                            pt[:], wgm_sb[:, c, bass.ts(mo, PC)], mhi[:, c, :],
                            start=(c == 0), stop=(c == KC - 1))
                    nc.vector.tensor_scalar_add(
                        mu32[:, mo, :], pt[:], bgm_sb[:, mo:mo + 1])
                if stage < 3:
                    yield (mu32, None)
                    return

                # ---- router logits (split fp16); the four kc chunks run
                # concurrently in the PE array's 32-column strips ----
                lg_ps = ps.tile([PC, tn], f32, tag="psm")
                for ti, (wt, rt) in enumerate(
                        ((wchi_sb, mhi), (wclo_sb, mhi), (wchi_sb, mlo))):
                    for c in range(KC):
                        nc.tensor.matmul(
                            lg_ps[32 * c:32 * c + K, :], wt[:, c, :],
                            rt[:, c, :], start=(ti == 0), stop=(ti == 2),
                            tile_position=(0, 32 * c),
                            skip_group_check=True)
                # gather the four 16-row partial strips onto partitions
                # 0-15 via DMA (cross-partition moves are DMA-only here),
                # then reduce with same-partition DVE adds
                lg4s = sp.tile([PC, tn], f32, tag="lg4s")
                for c in range(KC):
                    nc.vector.tensor_copy(lg4s[32 * c:32 * c + K, :],
                                          lg_ps[32 * c:32 * c + K, :])
                lg4 = sp.tile([K, KC, tn], f32, tag="lg4")
                for c in range(KC):
                    nc.sync.dma_start(lg4[:, c, :],
                                      lg4s[32 * c:32 * c + K, :])
                lga = sp.tile([K, tn], f32, tag="lga")
                nc.vector.tensor_tensor(lga[:], lg4[:, 0, :], lg4[:, 1, :],
                                        mybir.AluOpType.add)
                lgb = sp.tile([K, tn], f32, tag="lgb")
                nc.vector.tensor_tensor(lgb[:], lg4[:, 2, :], lg4[:, 3, :],
                                        mybir.AluOpType.add)
                lg = sp.tile([K, tn], f32, tag="lg")
                nc.vector.scalar_tensor_tensor(
                    lg[:], lga[:], bc_sb[:, 0:1], lgb[:],
                    op0=mybir.AluOpType.add, op1=mybir.AluOpType.add)
                yield (mu32, lg)

            def select(t, state):
                """Generator: yields between PE-visit stages."""
                if state is None or stage < 3:
                    return
                mu32, lg = state
                tsl = slice(t * tn, (t + 1) * tn)
                # ---- argmax: transpose logits to batch-major (exact fp32
                # identity matmul), rowwise max + is_equal, transpose the
                # 0/1 one-hot back (exact fp16 identity matmul) ----
                nch = tn // PC
                lgt_ps = ps.tile([PC, nch, K], f32, tag="psm")
                for c in range(nch):
                    nc.tensor.matmul(lgt_ps[:, c, :], lg[:, bass.ts(c, PC)],
                                     i16f_sb[:], start=(c == 0),
                                     stop=(c == nch - 1))
                yield
                # per-row argmax (first-match exact): min over e of
                # (e + BIG - BIG*[lg==max]); rebuild a guaranteed single-hot
                # mask from the index. All values are exact small ints.
                ohf_bm = sp.tile([PC, nch, K], f16, tag="ohbm")
                y_f = sp.tile([PC, nch], f32, tag="yf")
                for c in range(nch):
                    lmax = sp.tile([PC, 1], f32, tag="lmax")
                    nc.vector.tensor_reduce(
                        lmax[:], lgt_ps[:, c, :], axis=mybir.AxisListType.X,
                        op=mybir.AluOpType.max)
                    oh_c = sp.tile([PC, K], f16, tag="ohc")
                    nc.vector.tensor_scalar(
                        oh_c[:], lgt_ps[:, c, :], lmax[:, 0:1], None,
                        op0=mybir.AluOpType.is_equal)
                    val = sp.tile([PC, K], f32, tag="val")
                    nc.vector.scalar_tensor_tensor(
                        val[:], oh_c[:], -16.0, idxb_sb[:],
                        op0=mybir.AluOpType.mult, op1=mybir.AluOpType.add)
                    nc.vector.tensor_reduce(
                        y_f[:, c:c + 1], val[:], axis=mybir.AxisListType.X,
                        op=mybir.AluOpType.min)
                    nc.vector.tensor_scalar(
                        ohf_bm[:, c, :], idxc_sb[:], y_f[:, c:c + 1], None,
                        op0=mybir.AluOpType.is_equal)
                y_sb = sp.tile([PC, nch], i32, tag="ysb")
                nc.vector.tensor_copy(y_sb[:], y_f[:])
                if stage >= 4 and stage != 41:
                    nc.sync.dma_start(
                        y_d[tsl].rearrange("(c p) -> p c", p=PC), y_sb[:])
                yield
                ohf_ps = ps.tile([K, tn], f32, tag="psm")
                for c in range(nch):
                    nc.tensor.matmul(ohf_ps[:, bass.ts(c, PC)],
                                     ohf_bm[:, c, :], i128_sb[:],
                                     start=(c == 0), stop=(c == nch - 1))
                ohf = sp.tile([K, tn], f16, tag="ohf")
                nc.vector.tensor_copy(ohf[:], ohf_ps[:])
                yield
                if stage < 5:
                    return

                # ---- select expert mean: z = R^T (mu * (E^T ohf)) ----
                z_ps = ps.tile([K, tn], f32, tag="psm")
                for mo, e_sb in ((0, e0_sb), (1, e1_sb)):
                    msk_ps = pb.tile([PC, tn], f32, tag="pbig")
                    nc.tensor.matmul(msk_ps[:], e_sb[:], ohf[:],
                                     start=True, stop=True)
                    s_sb = sp.tile([PC, tn], f16, tag=f"s{mo}")
                    nc.vector.tensor_tensor(s_sb[:], mu32[:, mo, :], msk_ps[:],
                                            mybir.AluOpType.mult)
                    nc.tensor.matmul(z_ps[:], r_sb[:], s_sb[:],
                                     start=(mo == 0), stop=(mo == 1))
                z_sb = sp.tile([K, tn], f32, tag="zsb")
                nc.vector.tensor_copy(z_sb[:], z_ps[:])
                nc.sync.dma_start(z_d[:, tsl], z_sb[:])

            # Software pipeline: tile t-1's select stages (short PE visits
            # separated by DVE hops on tiny tensors) are interleaved between
            # tile t's PE-dense layer blocks, so the PE's in-order queue
            # always has dense work while each select stage's DVE inputs
            # complete.
            def drain(g):
                if g is not None:
                    for _ in g:
                        pass

            sels = {}
            for t in range(nt):
                pg = produce(t)
                next(pg)                      # L0
                if sels.get(t - 2) is not None:
                    drain(sels.pop(t - 2))    # stage C of t-2 (ohf ready)
                if sels.get(t - 1) is not None:
                    next(sels[t - 1], None)   # stage A of t-1 (argmax tr.)
                next(pg)                      # L1 first half
                if sels.get(t - 1) is not None:
                    next(sels[t - 1], None)   # stage A2 of t-1 (y / one-hot)
                state = next(pg)              # L1 rest + experts + router
                if sels.get(t - 1) is not None:
                    next(sels[t - 1], None)   # stage B of t-1 (ohf to fm)
                sels[t] = select(t, state)
            for t in sorted(sels):
                drain(sels[t])

    nc.compile()
    return nc


def _host_prep(x, W0, b0, W1, b1, W2, b2, Wc, bc, Wg, bg, bl=BL):
    """Build per-core input maps (numpy only)."""
    f16, f32 = np.float16, np.float32

    def split(a):
        hi = a.astype(f16)
        lo = (a.astype(f32) - hi.astype(f32)).astype(f16)
        return hi, lo

    xT = np.ascontiguousarray(np.asarray(x, f32).T)          # [D, B]
    xhi, xlo = split(xT)

    shared = {}
    for li, W in enumerate((W0, W1)):
        wT = np.ascontiguousarray(np.asarray(W, f32).T)       # [in, out]
        hi, lo = split(wT)
        shared[f"w{li}hi"] = hi
        shared[f"w{li}lo"] = lo
    # Fuse the identity-output layer W2 into the router and the experts
    # (host-side, fp64): logits = (Wc W2) h1 + (Wc b2 + bc),
    # mu = (Wgm W2) h1 + (Wgm b2 + bgm).
    f64 = np.float64
    W2_, b2_ = np.asarray(W2, f64), np.asarray(b2, f64)
    wfc = np.asarray(Wc, f64) @ W2_                            # [K, D]
    bfc = np.asarray(Wc, f64) @ b2_ + np.asarray(bc, f64)
    wgm2 = np.asarray(Wg, f64)[:, :LAT, :].reshape(K * LAT, D)
    wfg = wgm2 @ W2_                                           # [MU, D]
    bfg = wgm2 @ b2_ + np.asarray(bg, f64)[:, :LAT].reshape(-1)
    shared["wchi"], shared["wclo"] = split(
        np.ascontiguousarray(wfc.T).astype(f32))
    shared["wgm"] = np.ascontiguousarray(wfg.T).astype(f16)
    shared["b0"] = np.asarray(b0, f32)
    shared["b1"] = np.asarray(b1, f32)
    shared["bgm"] = bfg.astype(f32)
    shared["bc"] = bfc.astype(f32)

    kk = np.arange(K)
    pp = np.arange(PC)
    e0 = (pp[None, :] // LAT == kk[:, None]).astype(f16)
    e0[8:, :] = 0
    e1 = ((pp[None, :] // LAT + 8) == kk[:, None]).astype(f16)
    shared["e0"] = e0
    shared["e1"] = e1
    shared["r"] = (pp[:, None] % LAT == np.arange(LAT)[None, :]).astype(f16)
    shared["i16f"] = np.eye(K, dtype=f32)
    shared["i128"] = np.eye(PC, dtype=f16)
    shared["idxb"] = np.broadcast_to(kk.astype(f32) + 16.0, (PC, K)).copy()
    shared["idxc"] = np.broadcast_to(kk.astype(f32), (PC, K)).copy()

    ncores = xhi.shape[1] // bl
    in_maps = []
    for c in range(ncores):
        m = dict(shared)
        m["xhi"] = np.ascontiguousarray(xhi[:, c * bl:(c + 1) * bl])
        m["xlo"] = np.ascontiguousarray(xlo[:, c * bl:(c + 1) * bl])
        in_maps.append(m)
    return in_maps


def kernel(x, W0, b0, W1, b1, W2, b2, Wc, bc, Wg, bg, trace=False):
    from concourse.bass_utils import run_bass_kernel_spmd

    if "nc" not in _CACHE:
        _CACHE["nc"] = build_program()
    nc = _CACHE["nc"]

    in_maps = _host_prep(x, W0, b0, W1, b1, W2, b2, Wc, bc, Wg, bg)
    res = run_bass_kernel_spmd(nc, in_maps, core_ids=list(range(NCORES)),
                               trace=trace)
    z = np.concatenate(
        [np.ascontiguousarray(res.results[c]["z"].T) for c in range(NCORES)],
        axis=0)
    y = np.concatenate([res.results[c]["y"] for c in range(NCORES)], axis=0)
    if trace:
        kernel.last_results = res
    return z, y.astype(np.int32)


kernel.last_results = None


# revision 56
# speedup vs baseline: 1.0344x; 1.0084x over previous
"""Trainium2 Bass kernel for nn_EncoderMixtureModelTrajectory.

Model: 3-layer ReLU MLP (512->512->512->512) -> softmax router over 16
classes (argmax only is needed) -> per-class gaussian expert means
(mu = first 16 of 32 outputs), gather selected class's mu per sample.

Strategy: pure data-parallel over 8 NeuronCores (batch 65536 -> 8192/core).
Feature-major activations [D, N] on-chip; host pre-transposes x and weights.
Precision: fp16 hi/lo split matmuls (3 fp16 matmuls/pair ~ fp32 accuracy)
for the m-chain + router (the argmax decision is precision-critical);
single fp16 for the expert layer and selection machinery.
Router argmax + expert selection are done with small matmuls on the PE
(max-tree on DVE, one-hot via is_equal, first-match via triangular matmul,
select via expand/contract 0-1 matrices, batch-major transpose via identity
matmul).
"""

import os
import sys

if "/opt/trn_rl_repo" not in sys.path:
    sys.path.insert(0, "/opt/trn_rl_repo")
# Recover automatically if a previous run left a NeuronCore wedged.
os.environ.setdefault("NEURON_RT_RESET_CORES", "1")

import numpy as np

# Problem constants (hardcoded per task contract)
B = 65536
D = 512
LAT = 16
K = 16
NCORES = 8
BL = B // NCORES          # 8192 samples per core
TN = 512                  # batch-tile (free dim per matmul)
PC = 128                  # partitions
KC = D // PC              # 4 contraction chunks
MO = D // PC              # 4 output chunks for D-wide layers
MU = K * LAT              # 256 expert-mean rows
MUC = MU // PC            # 2 chunks

_CACHE = {}


def _dt():
    import concourse.mybir as mybir
    return mybir.dt


def build_program(bl=BL, tn=TN, stage=5):
    """Build the Bass/Tile program for one core (SPMD across 8).

    stage: debug bisection level (5 = full kernel)."""
    import concourse.bass as bass
    import concourse.bacc as bacc
    import concourse.mybir as mybir
    import concourse.tile as tile

    dt = mybir.dt
    f16, f32, i32 = dt.float16, dt.float32, dt.int32
    nt = bl // tn

    nc = bacc.Bacc("TRN2", target_bir_lowering=False, debug=False,
                   num_devices=NCORES)

    def din(name, shape, dtype):
        return nc.dram_tensor(name, shape, dtype, kind="ExternalInput").ap()

    def dout(name, shape, dtype):
        return nc.dram_tensor(name, shape, dtype, kind="ExternalOutput").ap()

    xhi_d = din("xhi", [D, bl], f16)
    xlo_d = din("xlo", [D, bl], f16)
    w_d = {}
    for li in range(2):
        for part in ("hi", "lo"):
            w_d[(li, part)] = din(f"w{li}{part}", [D, D], f16)
    wchi_d = din("wchi", [D, K], f16)
    wclo_d = din("wclo", [D, K], f16)
    wgm_d = din("wgm", [D, MU], f16)
    b_d = [din(f"b{li}", [D], f32) for li in range(2)]
    bgm_d = din("bgm", [MU], f32)
    bc_d = din("bc", [K], f32)
    e0_d = din("e0", [K, PC], f16)
    e1_d = din("e1", [K, PC], f16)
    r_d = din("r", [PC, K], f16)
    i16f_d = din("i16f", [K, K], f32)
    i128_d = din("i128", [PC, PC], f16)
    idxb_d = din("idxb", [PC, K], f32)
    idxc_d = din("idxc", [PC, K], f32)

    z_d = dout("z", [LAT, bl], f32)
    y_d = dout("y", [bl], i32)

    with tile.TileContext(nc) as tc:
        with (
            tc.tile_pool(name="wpool", bufs=1) as wp,
            tc.tile_pool(name="xpool", bufs=3) as xp,
            tc.tile_pool(name="hpool", bufs=3) as hp,
            tc.tile_pool(name="iopool", bufs=4) as iop,
            tc.tile_pool(name="spool", bufs=3) as sp,
            tc.tile_pool(name="pbig", bufs=4, space="PSUM") as pb,
            tc.tile_pool(name="psmall", bufs=4, space="PSUM") as ps,
        ):
            # ---- resident weights/constants ----
            # w0 (needed first) goes on the sync queue; everything else on
            # the gpsimd queue so tile-0's x/w0 DMAs aren't queued behind it.
            xhi_r = xhi_d.rearrange("(c p) n -> p c n", p=PC)
            xlo_r = xlo_d.rearrange("(c p) n -> p c n", p=PC)
            xhi0 = xp.tile([PC, KC, tn], f16, tag="xhi")
            xlo0 = xp.tile([PC, KC, tn], f16, tag="xlo")
            nc.sync.dma_start(xhi0[:], xhi_r[:, :, 0:tn])
            nc.gpsimd.dma_start(xlo0[:], xlo_r[:, :, 0:tn])
            w_sb = {}
            for key, d in w_d.items():
                t = wp.tile([PC, KC, D], f16, tag=f"w{key[0]}{key[1]}")
                w_sb[key] = t
            # w0 lands mo-chunk-interleaved so L0's first group starts early
            for mo in range(MO):
                for part in ("hi", "lo"):
                    nc.sync.dma_start(
                        w_sb[(0, part)][:, :, bass.ts(mo, PC)],
                        w_d[(0, part)].rearrange(
                            "(c p) o -> p c o", p=PC)[:, :, bass.ts(mo, PC)])
            for part in ("hi", "lo"):
                nc.gpsimd.dma_start(
                    w_sb[(1, part)][:],
                    w_d[(1, part)].rearrange("(c p) o -> p c o", p=PC))
            wchi_sb = wp.tile([PC, KC, K], f16, tag="wchi")
            nc.gpsimd.dma_start(wchi_sb[:], wchi_d.rearrange("(c p) o -> p c o", p=PC))
            wclo_sb = wp.tile([PC, KC, K], f16, tag="wclo")
            nc.gpsimd.dma_start(wclo_sb[:], wclo_d.rearrange("(c p) o -> p c o", p=PC))
            wgm_sb = wp.tile([PC, KC, MU], f16, tag="wgm")
            nc.gpsimd.dma_start(wgm_sb[:], wgm_d.rearrange("(c p) o -> p c o", p=PC))

            b_sb = []
            for li in range(2):
                t = wp.tile([PC, MO], f32, tag=f"b{li}")
                nc.gpsimd.dma_start(t[:], b_d[li].rearrange("(c p) -> p c", p=PC))
                b_sb.append(t)
            bgm_sb = wp.tile([PC, MUC], f32, tag="bgm")
            nc.gpsimd.dma_start(bgm_sb[:], bgm_d.rearrange("(c p) -> p c", p=PC))
            bc_sb = wp.tile([K, 1], f32, tag="bc")
            nc.gpsimd.dma_start(bc_sb[:], bc_d.rearrange("(k o) -> k o", o=1))
            e0_sb = wp.tile([K, PC], f16, tag="e0")
            nc.gpsimd.dma_start(e0_sb[:], e0_d[:])
            e1_sb = wp.tile([K, PC], f16, tag="e1")
            nc.gpsimd.dma_start(e1_sb[:], e1_d[:])
            r_sb = wp.tile([PC, K], f16, tag="r")
            nc.gpsimd.dma_start(r_sb[:], r_d[:])
            idxb_sb = wp.tile([PC, K], f32, tag="idxb")
            nc.gpsimd.dma_start(idxb_sb[:], idxb_d[:])
            idxc_sb = wp.tile([PC, K], f32, tag="idxc")
            nc.gpsimd.dma_start(idxc_sb[:], idxc_d[:])
            i16f_sb = wp.tile([K, K], f32, tag="i16f")
            nc.gpsimd.dma_start(i16f_sb[:], i16f_d[:])
            i128_sb = wp.tile([PC, PC], f16, tag="i128")
            nc.gpsimd.dma_start(i128_sb[:], i128_d[:])


            def split_layer(rhs_hi, rhs_lo, li, relu):
                """rhs_{hi,lo}: [PC, KC, tn] f16 -> returns (hi, lo) fp16
                tiles [PC, MO, tn] of relu(W x + b).

                hhi comes straight off the ACT engine (fp16 write rounds);
                the fp32 relu and the residual are computed on DVE in
                parallel, so the next layer's hi-term matmuls (issued
                first) only wait one ACT hop."""
                assert relu
                h32 = hp.tile([PC, MO, tn], f32, tag="h32")
                hhi = iop.tile([PC, MO, tn], f16, tag="hhi")
                hlo = iop.tile([PC, MO, tn], f16, tag="hlo")
                whi, wlo = w_sb[(li, "hi")], w_sb[(li, "lo")]
                for mo in range(MO):
                    pt = pb.tile([PC, tn], f32, tag="pbig")
                    n3 = 3 * KC
                    i = 0
                    for wt, rt in ((whi, rhs_hi), (wlo, rhs_hi), (whi, rhs_lo)):
                        for c in range(KC):
                            nc.tensor.matmul(
                                pt[:], wt[:, c, bass.ts(mo, PC)], rt[:, c, :],
                                start=(i == 0), stop=(i == n3 - 1))
                            i += 1
                    nc.scalar.activation(
                        h32[:, mo, :], pt[:],
                        mybir.ActivationFunctionType.Relu,
                        bias=b_sb[li][:, mo:mo + 1])
                    nc.scalar.copy(hhi[:, mo, :], h32[:, mo, :])
                    nc.vector.tensor_tensor(
                        hlo[:, mo, :], h32[:, mo, :], hhi[:, mo, :],
                        mybir.AluOpType.subtract)
                    if mo == 1:
                        yield (h32, hhi, hlo)
                yield (h32, hhi, hlo)

            def produce(t):
                """Layers + experts + router logits for tile t (PE-dense,
                shallow cross-engine chains). Generator: yields at PE-dense
                stage boundaries so select() stages of the previous tile can
                be interleaved; final value is the state for select()."""
                tsl = slice(t * tn, (t + 1) * tn)
                if t == 0:
                    xhi, xlo = xhi0, xlo0   # prefetched before the weights
                else:
                    xhi = xp.tile([PC, KC, tn], f16, tag="xhi")
                    xlo = xp.tile([PC, KC, tn], f16, tag="xlo")
                    nc.sync.dma_start(xhi[:], xhi_r[:, :, tsl])
                    nc.sync.dma_start(xlo[:], xlo_r[:, :, tsl])

                g0 = split_layer(xhi, xlo, 0, relu=True)
                next(g0)
                _, h0hi, h0lo = next(g0)
                yield None
                g1 = split_layer(h0hi, h0lo, 1, relu=True)
                next(g1)
                yield None
                h1_32, mhi, mlo = next(g1)
                if stage < 2:
                    yield None
                    return

                # ---- experts: mu = Wgm m + bgm (single fp16) ----
                mu32 = sp.tile([PC, MUC, tn], f32, tag="mu32")
                for mo in range(MUC):
                    pt = pb.tile([PC, tn], f32, tag="pbig")
                    for c in range(KC):
                        nc.tensor.matmul(
                            pt[:], wgm_sb[:, c, bass.ts(mo, PC)], mhi[:, c, :],
                            start=(c == 0), stop=(c == KC - 1))
                    nc.vector.tensor_scalar_add(
                        mu32[:, mo, :], pt[:], bgm_sb[:, mo:mo + 1])
                if stage < 3:
                    yield (mu32, None)
                    return

                # ---- router logits (split fp16); the four kc chunks run
                # concurrently in the PE array's 32-column strips ----
                lg_ps = ps.tile([PC, tn], f32, tag="psm")
                for ti, (wt, rt) in enumerate(
                        ((wchi_sb, mhi), (wclo_sb, mhi), (wchi_sb, mlo))):
                    for c in range(KC):
                        nc.tensor.matmul(
                            lg_ps[32 * c:32 * c + K, :], wt[:, c, :],
                            rt[:, c, :], start=(ti == 0), stop=(ti == 2),
                            tile_position=(0, 32 * c),
                            skip_group_check=True)
                # gather the four 16-row partial strips onto partitions
                # 0-15 via DMA (cross-partition moves are DMA-only here),
                # then reduce with same-partition DVE adds
                lg4s = sp.tile([PC, tn], f32, tag="lg4s")
                for c in range(KC):
                    nc.vector.tensor_copy(lg4s[32 * c:32 * c + K, :],
                                          lg_ps[32 * c:32 * c + K, :])
                lg4 = sp.tile([K, KC, tn], f32, tag="lg4")
                for c in range(KC):
                    nc.sync.dma_start(lg4[:, c, :],
                                      lg4s[32 * c:32 * c + K, :])
                lga = sp.tile([K, tn], f32, tag="lga")
                nc.vector.tensor_tensor(lga[:], lg4[:, 0, :], lg4[:, 1, :],
                                        mybir.AluOpType.add)
                lgb = sp.tile([K, tn], f32, tag="lgb")
                nc.vector.tensor_tensor(lgb[:], lg4[:, 2, :], lg4[:, 3, :],
                                        mybir.AluOpType.add)
                lg = sp.tile([K, tn], f32, tag="lg")
                nc.vector.scalar_tensor_tensor(
                    lg[:], lga[:], bc_sb[:, 0:1], lgb[:],
                    op0=mybir.AluOpType.add, op1=mybir.AluOpType.add)
                yield (mu32, lg)

            def select(t, state):
                """Generator: yields between PE-visit stages."""
                if state is None or stage < 3:
                    return
                mu32, lg = state
                tsl = slice(t * tn, (t + 1) * tn)
                # ---- argmax: transpose logits to batch-major (exact fp32
                # identity matmul), rowwise max + is_equal, transpose the
                # 0/1 one-hot back (exact fp16 identity matmul) ----
                nch = tn // PC
                lgt_ps = ps.tile([PC, nch, K], f32, tag="psm")
                for c in range(nch):
                    nc.tensor.matmul(lgt_ps[:, c, :], lg[:, bass.ts(c, PC)],
                                     i16f_sb[:], start=(c == 0),
                                     stop=(c == nch - 1))
                yield
                # per-row argmax (first-match exact): min over e of
                # (e + BIG - BIG*[lg==max]); rebuild a guaranteed single-hot
                # mask from the index. All values are exact small ints.
                ohf_bm = sp.tile([PC, nch, K], f16, tag="ohbm")
                y_f = sp.tile([PC, nch], f32, tag="yf")
                for c in range(nch):
                    lmax = sp.tile([PC, 1], f32, tag="lmax")
                    nc.vector.tensor_reduce(
                        lmax[:], lgt_ps[:, c, :], axis=mybir.AxisListType.X,
                        op=mybir.AluOpType.max)
                    oh_c = sp.tile([PC, K], f16, tag="ohc")
                    nc.vector.tensor_scalar(
                        oh_c[:], lgt_ps[:, c, :], lmax[:, 0:1], None,
                        op0=mybir.AluOpType.is_equal)
                    val = sp.tile([PC, K], f32, tag="val")
                    nc.vector.scalar_tensor_tensor(
                        val[:], oh_c[:], -16.0, idxb_sb[:],
                        op0=mybir.AluOpType.mult, op1=mybir.AluOpType.add)
                    nc.vector.tensor_reduce(
                        y_f[:, c:c + 1], val[:], axis=mybir.AxisListType.X,
                        op=mybir.AluOpType.min)
                    nc.vector.tensor_scalar(
                        ohf_bm[:, c, :], idxc_sb[:], y_f[:, c:c + 1], None,
                        op0=mybir.AluOpType.is_equal)
                y_sb = sp.tile([PC, nch], i32, tag="ysb")
                nc.vector.tensor_copy(y_sb[:], y_f[:])
                if stage >= 4 and stage != 41:
                    nc.sync.dma_start(
                        y_d[tsl].rearrange("(c p) -> p c", p=PC), y_sb[:])
                yield
                ohf_ps = ps.tile([K, tn], f32, tag="psm")
                for c in range(nch):
                    nc.tensor.matmul(ohf_ps[:, bass.ts(c, PC)],
                                     ohf_bm[:, c, :], i128_sb[:],
                                     start=(c == 0), stop=(c == nch - 1))
                ohf = sp.tile([K, tn], f16, tag="ohf")
                nc.vector.tensor_copy(ohf[:], ohf_ps[:])
                yield
                if stage < 5:
                    return

                # ---- select expert mean: z = R^T (mu * (E^T ohf)) ----
                z_ps = ps.tile([K, tn], f32, tag="psm")
                for mo, e_sb in ((0, e0_sb), (1, e1_sb)):
                    msk_ps = pb.tile([PC, tn], f32, tag="pbig")
                    nc.tensor.matmul(msk_ps[:], e_sb[:], ohf[:],
                                     start=True, stop=True)
                    s_sb = sp.tile([PC, tn], f16, tag=f"s{mo}")
                    nc.vector.tensor_tensor(s_sb[:], mu32[:, mo, :], msk_ps[:],
                                            mybir.AluOpType.mult)
                    nc.tensor.matmul(z_ps[:], r_sb[:], s_sb[:],
                                     start=(mo == 0), stop=(mo == 1))
                z_sb = sp.tile([K, tn], f32, tag="zsb")
                nc.vector.tensor_copy(z_sb[:], z_ps[:])
                nc.sync.dma_start(z_d[:, tsl], z_sb[:])

            # Software pipeline: tile t-1's select stages (short PE visits
            # separated by DVE hops on tiny tensors) are interleaved between
            # tile t's PE-dense layer blocks, so the PE's in-order queue
            # always has dense work while each select stage's DVE inputs
            # complete.
            def drain(g):
                if g is not None:
                    for _ in g:
                        pass

            sels = {}
            for t in range(nt):
                pg = produce(t)
                next(pg)                      # L0
                if sels.get(t - 2) is not None:
                    drain(sels.pop(t - 2))    # stage C of t-2 (ohf ready)
                if sels.get(t - 1) is not None:
                    next(sels[t - 1], None)   # stage A of t-1 (argmax tr.)
                next(pg)                      # L1 first half
                if sels.get(t - 1) is not None:
                    next(sels[t - 1], None)   # stage A2 of t-1 (y / one-hot)
                state = next(pg)              # L1 rest + experts + router
                if sels.get(t - 1) is not None:
                    next(sels[t - 1], None)   # stage B of t-1 (ohf to fm)
                sels[t] = select(t, state)
            for t in sorted(sels):
                drain(sels[t])

    nc.compile()
    return nc


def _host_prep(x, W0, b0, W1, b1, W2, b2, Wc, bc, Wg, bg, bl=BL):
    """Build per-core input maps (numpy only)."""
    f16, f32 = np.float16, np.float32

    def split(a):
        hi = a.astype(f16)
        lo = (a.astype(f32) - hi.astype(f32)).astype(f16)
        return hi, lo

    xT = np.ascontiguousarray(np.asarray(x, f32).T)          # [D, B]
    xhi, xlo = split(xT)

    shared = {}
    for li, W in enumerate((W0, W1)):
        wT = np.ascontiguousarray(np.asarray(W, f32).T)       # [in, out]
        hi, lo = split(wT)
        shared[f"w{li}hi"] = hi
        shared[f"w{li}lo"] = lo
    # Fuse the identity-output layer W2 into the router and the experts
    # (host-side, fp64): logits = (Wc W2) h1 + (Wc b2 + bc),
    # mu = (Wgm W2) h1 + (Wgm b2 + bgm).
    f64 = np.float64
    W2_, b2_ = np.asarray(W2, f64), np.asarray(b2, f64)
    wfc = np.asarray(Wc, f64) @ W2_                            # [K, D]
    bfc = np.asarray(Wc, f64) @ b2_ + np.asarray(bc, f64)
    wgm2 = np.asarray(Wg, f64)[:, :LAT, :].reshape(K * LAT, D)
    wfg = wgm2 @ W2_                                           # [MU, D]
    bfg = wgm2 @ b2_ + np.asarray(bg, f64)[:, :LAT].reshape(-1)
    shared["wchi"], shared["wclo"] = split(
        np.ascontiguousarray(wfc.T).astype(f32))
    shared["wgm"] = np.ascontiguousarray(wfg.T).astype(f16)
    shared["b0"] = np.asarray(b0, f32)
    shared["b1"] = np.asarray(b1, f32)
    shared["bgm"] = bfg.astype(f32)
    shared["bc"] = bfc.astype(f32)

    kk = np.arange(K)
    pp = np.arange(PC)
    e0 = (pp[None, :] // LAT == kk[:, None]).astype(f16)
    e0[8:, :] = 0
    e1 = ((pp[None, :] // LAT + 8) == kk[:, None]).astype(f16)
    shared["e0"] = e0
    shared["e1"] = e1
    shared["r"] = (pp[:, None] % LAT == np.arange(LAT)[None, :]).astype(f16)
    shared["i16f"] = np.eye(K, dtype=f32)
    shared["i128"] = np.eye(PC, dtype=f16)
    shared["idxb"] = np.broadcast_to(kk.astype(f32) + 16.0, (PC, K)).copy()
    shared["idxc"] = np.broadcast_to(kk.astype(f32), (PC, K)).copy()

    ncores = xhi.shape[1] // bl
    in_maps = []
    for c in range(ncores):
        m = dict(shared)
        m["xhi"] = np.ascontiguousarray(xhi[:, c * bl:(c + 1) * bl])
        m["xlo"] = np.ascontiguousarray(xlo[:, c * bl:(c + 1) * bl])
        in_maps.append(m)
    return in_maps


def kernel(x, W0, b0, W1, b1, W2, b2, Wc, bc, Wg, bg, trace=False):
    from concourse.bass_utils import run_bass_kernel_spmd

    if "nc" not in _CACHE:
        _CACHE["nc"] = build_program()
    nc = _CACHE["nc"]

    in_maps = _host_prep(x, W0, b0, W1, b1, W2, b2, Wc, bc, Wg, bg)
    res = run_bass_kernel_spmd(nc, in_maps, core_ids=list(range(NCORES)),
                               trace=trace)
    z = np.concatenate(
        [np.ascontiguousarray(res.results[c]["z"].T) for c in range(NCORES)],
        axis=0)
    y = np.concatenate([res.results[c]["y"] for c in range(NCORES)], axis=0)
    if trace:
        kernel.last_results = res
    return z, y.astype(np.int32)


kernel.last_results = None


# revision 57
# speedup vs baseline: 1.0377x; 1.0032x over previous
"""Trainium2 Bass kernel for nn_EncoderMixtureModelTrajectory.

Model: 3-layer ReLU MLP (512->512->512->512) -> softmax router over 16
classes (argmax only is needed) -> per-class gaussian expert means
(mu = first 16 of 32 outputs), gather selected class's mu per sample.

Strategy: pure data-parallel over 8 NeuronCores (batch 65536 -> 8192/core).
Feature-major activations [D, N] on-chip; host pre-transposes x and weights.
Precision: fp16 hi/lo split matmuls (3 fp16 matmuls/pair ~ fp32 accuracy)
for the m-chain + router (the argmax decision is precision-critical);
single fp16 for the expert layer and selection machinery.
Router argmax + expert selection are done with small matmuls on the PE
(max-tree on DVE, one-hot via is_equal, first-match via triangular matmul,
select via expand/contract 0-1 matrices, batch-major transpose via identity
matmul).
"""

import os
import sys

if "/opt/trn_rl_repo" not in sys.path:
    sys.path.insert(0, "/opt/trn_rl_repo")
# Recover automatically if a previous run left a NeuronCore wedged.
os.environ.setdefault("NEURON_RT_RESET_CORES", "1")

import numpy as np

# Problem constants (hardcoded per task contract)
B = 65536
D = 512
LAT = 16
K = 16
NCORES = 8
BL = B // NCORES          # 8192 samples per core
TN = 512                  # batch-tile (free dim per matmul)
PC = 128                  # partitions
KC = D // PC              # 4 contraction chunks
MO = D // PC              # 4 output chunks for D-wide layers
MU = K * LAT              # 256 expert-mean rows
MUC = MU // PC            # 2 chunks

_CACHE = {}


def _dt():
    import concourse.mybir as mybir
    return mybir.dt


def build_program(bl=BL, tn=TN, stage=5):
    """Build the Bass/Tile program for one core (SPMD across 8).

    stage: debug bisection level (5 = full kernel)."""
    import concourse.bass as bass
    import concourse.bacc as bacc
    import concourse.mybir as mybir
    import concourse.tile as tile

    dt = mybir.dt
    f16, f32, i32 = dt.float16, dt.float32, dt.int32
    nt = bl // tn

    nc = bacc.Bacc("TRN2", target_bir_lowering=False, debug=False,
                   num_devices=NCORES)

    def din(name, shape, dtype):
        return nc.dram_tensor(name, shape, dtype, kind="ExternalInput").ap()

    def dout(name, shape, dtype):
        return nc.dram_tensor(name, shape, dtype, kind="ExternalOutput").ap()

    xhi_d = din("xhi", [D, bl], f16)
    xlo_d = din("xlo", [D, bl], f16)
    w_d = {}
    for li in range(2):
        for part in ("hi", "lo"):
            w_d[(li, part)] = din(f"w{li}{part}", [D, D], f16)
    wchi_d = din("wchi", [D, K], f16)
    wclo_d = din("wclo", [D, K], f16)
    wgm_d = din("wgm", [D, MU], f16)
    b_d = [din(f"b{li}", [D], f32) for li in range(2)]
    bgm_d = din("bgm", [MU], f32)
    bc_d = din("bc", [K], f32)
    e0_d = din("e0", [K, PC], f16)
    e1_d = din("e1", [K, PC], f16)
    r_d = din("r", [PC, K], f16)
    i16f_d = din("i16f", [K, K], f32)
    i128_d = din("i128", [PC, PC], f16)
    idxb_d = din("idxb", [PC, K], f32)
    idxc_d = din("idxc", [PC, K], f32)

    z_d = dout("z", [LAT, bl], f32)
    y_d = dout("y", [bl], i32)

    with tile.TileContext(nc) as tc:
        with (
            tc.tile_pool(name="wpool", bufs=1) as wp,
            tc.tile_pool(name="xpool", bufs=3) as xp,
            tc.tile_pool(name="hpool", bufs=3) as hp,
            tc.tile_pool(name="iopool", bufs=4) as iop,
            tc.tile_pool(name="spool", bufs=3) as sp,
            tc.tile_pool(name="pbig", bufs=4, space="PSUM") as pb,
            tc.tile_pool(name="psmall", bufs=4, space="PSUM") as ps,
        ):
            # ---- resident weights/constants ----
            # w0 (needed first) goes on the sync queue; everything else on
            # the gpsimd queue so tile-0's x/w0 DMAs aren't queued behind it.
            xhi_r = xhi_d.rearrange("(c p) n -> p c n", p=PC)
            xlo_r = xlo_d.rearrange("(c p) n -> p c n", p=PC)
            xhi0 = xp.tile([PC, KC, tn], f16, tag="xhi")
            xlo0 = xp.tile([PC, KC, tn], f16, tag="xlo")
            nc.sync.dma_start(xhi0[:, 0:2, :], xhi_r[:, 0:2, 0:tn])
            nc.gpsimd.dma_start(xhi0[:, 2:4, :], xhi_r[:, 2:4, 0:tn])
            nc.gpsimd.dma_start(xlo0[:, 0:2, :], xlo_r[:, 0:2, 0:tn])
            nc.sync.dma_start(xlo0[:, 2:4, :], xlo_r[:, 2:4, 0:tn])
            w_sb = {}
            for key, d in w_d.items():
                t = wp.tile([PC, KC, D], f16, tag=f"w{key[0]}{key[1]}")
                w_sb[key] = t
            # w0 lands mo-chunk-interleaved so L0's first group starts early
            for mo in range(MO):
                for part in ("hi", "lo"):
                    nc.sync.dma_start(
                        w_sb[(0, part)][:, :, bass.ts(mo, PC)],
                        w_d[(0, part)].rearrange(
                            "(c p) o -> p c o", p=PC)[:, :, bass.ts(mo, PC)])
            for part in ("hi", "lo"):
                nc.gpsimd.dma_start(
                    w_sb[(1, part)][:],
                    w_d[(1, part)].rearrange("(c p) o -> p c o", p=PC))
            wchi_sb = wp.tile([PC, KC, K], f16, tag="wchi")
            nc.gpsimd.dma_start(wchi_sb[:], wchi_d.rearrange("(c p) o -> p c o", p=PC))
            wclo_sb = wp.tile([PC, KC, K], f16, tag="wclo")
            nc.gpsimd.dma_start(wclo_sb[:], wclo_d.rearrange("(c p) o -> p c o", p=PC))
            wgm_sb = wp.tile([PC, KC, MU], f16, tag="wgm")
            nc.gpsimd.dma_start(wgm_sb[:], wgm_d.rearrange("(c p) o -> p c o", p=PC))

            b_sb = []
            for li in range(2):
                t = wp.tile([PC, MO], f32, tag=f"b{li}")
                nc.gpsimd.dma_start(t[:], b_d[li].rearrange("(c p) -> p c", p=PC))
                b_sb.append(t)
            bgm_sb = wp.tile([PC, MUC], f32, tag="bgm")
            nc.gpsimd.dma_start(bgm_sb[:], bgm_d.rearrange("(c p) -> p c", p=PC))
            bc_sb = wp.tile([K, 1], f32, tag="bc")
            nc.gpsimd.dma_start(bc_sb[:], bc_d.rearrange("(k o) -> k o", o=1))
            e0_sb = wp.tile([K, PC], f16, tag="e0")
            nc.gpsimd.dma_start(e0_sb[:], e0_d[:])
            e1_sb = wp.tile([K, PC], f16, tag="e1")
            nc.gpsimd.dma_start(e1_sb[:], e1_d[:])
            r_sb = wp.tile([PC, K], f16, tag="r")
            nc.gpsimd.dma_start(r_sb[:], r_d[:])
            idxb_sb = wp.tile([PC, K], f32, tag="idxb")
            nc.gpsimd.dma_start(idxb_sb[:], idxb_d[:])
            idxc_sb = wp.tile([PC, K], f32, tag="idxc")
            nc.gpsimd.dma_start(idxc_sb[:], idxc_d[:])
            i16f_sb = wp.tile([K, K], f32, tag="i16f")
            nc.gpsimd.dma_start(i16f_sb[:], i16f_d[:])
            i128_sb = wp.tile([PC, PC], f16, tag="i128")
            nc.gpsimd.dma_start(i128_sb[:], i128_d[:])


            def split_layer(rhs_hi, rhs_lo, li, relu):
                """rhs_{hi,lo}: [PC, KC, tn] f16 -> returns (hi, lo) fp16
                tiles [PC, MO, tn] of relu(W x + b).

                hhi comes straight off the ACT engine (fp16 write rounds);
                the fp32 relu and the residual are computed on DVE in
                parallel, so the next layer's hi-term matmuls (issued
                first) only wait one ACT hop."""
                assert relu
                h32 = hp.tile([PC, MO, tn], f32, tag="h32")
                hhi = iop.tile([PC, MO, tn], f16, tag="hhi")
                hlo = iop.tile([PC, MO, tn], f16, tag="hlo")
                whi, wlo = w_sb[(li, "hi")], w_sb[(li, "lo")]
                for mo in range(MO):
                    pt = pb.tile([PC, tn], f32, tag="pbig")
                    n3 = 3 * KC
                    i = 0
                    for wt, rt in ((whi, rhs_hi), (wlo, rhs_hi), (whi, rhs_lo)):
                        for c in range(KC):
                            nc.tensor.matmul(
                                pt[:], wt[:, c, bass.ts(mo, PC)], rt[:, c, :],
                                start=(i == 0), stop=(i == n3 - 1))
                            i += 1
                    nc.scalar.activation(
                        h32[:, mo, :], pt[:],
                        mybir.ActivationFunctionType.Relu,
                        bias=b_sb[li][:, mo:mo + 1])
                    nc.scalar.copy(hhi[:, mo, :], h32[:, mo, :])
                    nc.vector.tensor_tensor(
                        hlo[:, mo, :], h32[:, mo, :], hhi[:, mo, :],
                        mybir.AluOpType.subtract)
                    if mo == 1:
                        yield (h32, hhi, hlo)
                yield (h32, hhi, hlo)

            def produce(t):
                """Layers + experts + router logits for tile t (PE-dense,
                shallow cross-engine chains). Generator: yields at PE-dense
                stage boundaries so select() stages of the previous tile can
                be interleaved; final value is the state for select()."""
                tsl = slice(t * tn, (t + 1) * tn)
                if t == 0:
                    xhi, xlo = xhi0, xlo0   # prefetched before the weights
                else:
                    xhi = xp.tile([PC, KC, tn], f16, tag="xhi")
                    xlo = xp.tile([PC, KC, tn], f16, tag="xlo")
                    nc.sync.dma_start(xhi[:], xhi_r[:, :, tsl])
                    nc.sync.dma_start(xlo[:], xlo_r[:, :, tsl])

                g0 = split_layer(xhi, xlo, 0, relu=True)
                next(g0)
                _, h0hi, h0lo = next(g0)
                yield None
                g1 = split_layer(h0hi, h0lo, 1, relu=True)
                next(g1)
                yield None
                h1_32, mhi, mlo = next(g1)
                if stage < 2:
                    yield None
                    return

                # ---- experts: mu = Wgm m + bgm (single fp16) ----
                mu32 = sp.tile([PC, MUC, tn], f32, tag="mu32")
                for mo in range(MUC):
                    pt = pb.tile([PC, tn], f32, tag="pbig")
                    for c in range(KC):
                        nc.tensor.matmul(
                            pt[:], wgm_sb[:, c, bass.ts(mo, PC)], mhi[:, c, :],
                            start=(c == 0), stop=(c == KC - 1))
                    nc.vector.tensor_scalar_add(
                        mu32[:, mo, :], pt[:], bgm_sb[:, mo:mo + 1])
                if stage < 3:
                    yield (mu32, None)
                    return

                # ---- router logits (split fp16); the four kc chunks run
                # concurrently in the PE array's 32-column strips ----
                lg_ps = ps.tile([PC, tn], f32, tag="psm")
                for ti, (wt, rt) in enumerate(
                        ((wchi_sb, mhi), (wclo_sb, mhi), (wchi_sb, mlo))):
                    for c in range(KC):
                        nc.tensor.matmul(
                            lg_ps[32 * c:32 * c + K, :], wt[:, c, :],
                            rt[:, c, :], start=(ti == 0), stop=(ti == 2),
                            tile_position=(0, 32 * c),
                            skip_group_check=True)
                # gather the four 16-row partial strips onto partitions
                # 0-15 via DMA (cross-partition moves are DMA-only here),
                # then reduce with same-partition DVE adds
                lg4s = sp.tile([PC, tn], f32, tag="lg4s")
                for c in range(KC):
                    nc.vector.tensor_copy(lg4s[32 * c:32 * c + K, :],
                                          lg_ps[32 * c:32 * c + K, :])
                lg4 = sp.tile([K, KC, tn], f32, tag="lg4")
                for c in range(KC):
                    nc.sync.dma_start(lg4[:, c, :],
                                      lg4s[32 * c:32 * c + K, :])
                lga = sp.tile([K, tn], f32, tag="lga")
                nc.vector.tensor_tensor(lga[:], lg4[:, 0, :], lg4[:, 1, :],
                                        mybir.AluOpType.add)
                lgb = sp.tile([K, tn], f32, tag="lgb")
                nc.vector.tensor_tensor(lgb[:], lg4[:, 2, :], lg4[:, 3, :],
                                        mybir.AluOpType.add)
                lg = sp.tile([K, tn], f32, tag="lg")
                nc.vector.scalar_tensor_tensor(
                    lg[:], lga[:], bc_sb[:, 0:1], lgb[:],
                    op0=mybir.AluOpType.add, op1=mybir.AluOpType.add)
                yield (mu32, lg)

            def select(t, state):
                """Generator: yields between PE-visit stages."""
                if state is None or stage < 3:
                    return
                mu32, lg = state
                tsl = slice(t * tn, (t + 1) * tn)
                # ---- argmax: transpose logits to batch-major (exact fp32
                # identity matmul), rowwise max + is_equal, transpose the
                # 0/1 one-hot back (exact fp16 identity matmul) ----
                nch = tn // PC
                lgt_ps = ps.tile([PC, nch, K], f32, tag="psm")
                for c in range(nch):
                    nc.tensor.matmul(lgt_ps[:, c, :], lg[:, bass.ts(c, PC)],
                                     i16f_sb[:], start=(c == 0),
                                     stop=(c == nch - 1))
                yield
                # per-row argmax (first-match exact): min over e of
                # (e + BIG - BIG*[lg==max]); rebuild a guaranteed single-hot
                # mask from the index. All values are exact small ints.
                ohf_bm = sp.tile([PC, nch, K], f16, tag="ohbm")
                y_f = sp.tile([PC, nch], f32, tag="yf")
                for c in range(nch):
                    lmax = sp.tile([PC, 1], f32, tag="lmax")
                    nc.vector.tensor_reduce(
                        lmax[:], lgt_ps[:, c, :], axis=mybir.AxisListType.X,
                        op=mybir.AluOpType.max)
                    oh_c = sp.tile([PC, K], f16, tag="ohc")
                    nc.vector.tensor_scalar(
                        oh_c[:], lgt_ps[:, c, :], lmax[:, 0:1], None,
                        op0=mybir.AluOpType.is_equal)
                    val = sp.tile([PC, K], f32, tag="val")
                    nc.vector.scalar_tensor_tensor(
                        val[:], oh_c[:], -16.0, idxb_sb[:],
                        op0=mybir.AluOpType.mult, op1=mybir.AluOpType.add)
                    nc.vector.tensor_reduce(
                        y_f[:, c:c + 1], val[:], axis=mybir.AxisListType.X,
                        op=mybir.AluOpType.min)
                    nc.vector.tensor_scalar(
                        ohf_bm[:, c, :], idxc_sb[:], y_f[:, c:c + 1], None,
                        op0=mybir.AluOpType.is_equal)
                y_sb = sp.tile([PC, nch], i32, tag="ysb")
                nc.vector.tensor_copy(y_sb[:], y_f[:])
                if stage >= 4 and stage != 41:
                    nc.sync.dma_start(
                        y_d[tsl].rearrange("(c p) -> p c", p=PC), y_sb[:])
                yield
                ohf_ps = ps.tile([K, tn], f32, tag="psm")
                for c in range(nch):
                    nc.tensor.matmul(ohf_ps[:, bass.ts(c, PC)],
                                     ohf_bm[:, c, :], i128_sb[:],
                                     start=(c == 0), stop=(c == nch - 1))
                ohf = sp.tile([K, tn], f16, tag="ohf")
                nc.vector.tensor_copy(ohf[:], ohf_ps[:])
                yield
                if stage < 5:
                    return

                # ---- select expert mean: z = R^T (mu * (E^T ohf)) ----
                z_ps = ps.tile([K, tn], f32, tag="psm")
                msks = []
                for mo, e_sb in ((0, e0_sb), (1, e1_sb)):
                    msk_ps = pb.tile([PC, tn], f32, tag="pbig")
                    nc.tensor.matmul(msk_ps[:], e_sb[:], ohf[:],
                                     start=True, stop=True)
                    msks.append(msk_ps)
                s_sbs = []
                for mo in range(MUC):
                    s_sb = sp.tile([PC, tn], f16, tag=f"s{mo}")
                    nc.vector.tensor_tensor(s_sb[:], mu32[:, mo, :],
                                            msks[mo][:],
                                            mybir.AluOpType.mult)
                    s_sbs.append(s_sb)
                for mo in range(MUC):
                    nc.tensor.matmul(z_ps[:], r_sb[:], s_sbs[mo][:],
                                     start=(mo == 0), stop=(mo == 1))
                z_sb = sp.tile([K, tn], f32, tag="zsb")
                nc.vector.tensor_copy(z_sb[:], z_ps[:])
                nc.sync.dma_start(z_d[:, tsl], z_sb[:])

            # Software pipeline: tile t-1's select stages (short PE visits
            # separated by DVE hops on tiny tensors) are interleaved between
            # tile t's PE-dense layer blocks, so the PE's in-order queue
            # always has dense work while each select stage's DVE inputs
            # complete.
            def drain(g):
                if g is not None:
                    for _ in g:
                        pass

            sels = {}
            for t in range(nt):
                pg = produce(t)
                next(pg)                      # L0
                if sels.get(t - 2) is not None:
                    drain(sels.pop(t - 2))    # stage C of t-2 (ohf ready)
                if sels.get(t - 1) is not None:
                    next(sels[t - 1], None)   # stage A of t-1 (argmax tr.)
                next(pg)                      # L1 first half
                if sels.get(t - 1) is not None:
                    next(sels[t - 1], None)   # stage A2 of t-1 (y / one-hot)
                state = next(pg)              # L1 rest + experts + router
                if sels.get(t - 1) is not None:
                    next(sels[t - 1], None)   # stage B of t-1 (ohf to fm)
                sels[t] = select(t, state)
            rest = [sels[t] for t in sorted(sels)]
            if len(rest) == 2:
                older, last = rest
                next(last, None)          # A of last tile
                drain(older)              # C of second-to-last fills the gap
                drain(last)               # A2/B/C of last
            else:
                for g in rest:
                    drain(g)

    nc.compile()
    return nc


def _host_prep(x, W0, b0, W1, b1, W2, b2, Wc, bc, Wg, bg, bl=BL):
    """Build per-core input maps (numpy only)."""
    f16, f32 = np.float16, np.float32

    def split(a):
        hi = a.astype(f16)
        lo = (a.astype(f32) - hi.astype(f32)).astype(f16)
        return hi, lo

    xT = np.ascontiguousarray(np.asarray(x, f32).T)          # [D, B]
    xhi, xlo = split(xT)

    shared = {}
    for li, W in enumerate((W0, W1)):
        wT = np.ascontiguousarray(np.asarray(W, f32).T)       # [in, out]
        hi, lo = split(wT)
        shared[f"w{li}hi"] = hi
        shared[f"w{li}lo"] = lo
    # Fuse the identity-output layer W2 into the router and the experts
    # (host-side, fp64): logits = (Wc W2) h1 + (Wc b2 + bc),
    # mu = (Wgm W2) h1 + (Wgm b2 + bgm).
    f64 = np.float64
    W2_, b2_ = np.asarray(W2, f64), np.asarray(b2, f64)
    wfc = np.asarray(Wc, f64) @ W2_                            # [K, D]
    bfc = np.asarray(Wc, f64) @ b2_ + np.asarray(bc, f64)
    wgm2 = np.asarray(Wg, f64)[:, :LAT, :].reshape(K * LAT, D)
    wfg = wgm2 @ W2_                                           # [MU, D]
    bfg = wgm2 @ b2_ + np.asarray(bg, f64)[:, :LAT].reshape(-1)
    shared["wchi"], shared["wclo"] = split(
        np.ascontiguousarray(wfc.T).astype(f32))
    shared["wgm"] = np.ascontiguousarray(wfg.T).astype(f16)
    shared["b0"] = np.asarray(b0, f32)
    shared["b1"] = np.asarray(b1, f32)
    shared["bgm"] = bfg.astype(f32)
    shared["bc"] = bfc.astype(f32)

    kk = np.arange(K)
    pp = np.arange(PC)
    e0 = (pp[None, :] // LAT == kk[:, None]).astype(f16)
    e0[8:, :] = 0
    e1 = ((pp[None, :] // LAT + 8) == kk[:, None]).astype(f16)
    shared["e0"] = e0
    shared["e1"] = e1
    shared["r"] = (pp[:, None] % LAT == np.arange(LAT)[None, :]).astype(f16)
    shared["i16f"] = np.eye(K, dtype=f32)
    shared["i128"] = np.eye(PC, dtype=f16)
    shared["idxb"] = np.broadcast_to(kk.astype(f32) + 16.0, (PC, K)).copy()
    shared["idxc"] = np.broadcast_to(kk.astype(f32), (PC, K)).copy()

    ncores = xhi.shape[1] // bl
    in_maps = []
    for c in range(ncores):
        m = dict(shared)
        m["xhi"] = np.ascontiguousarray(xhi[:, c * bl:(c + 1) * bl])
        m["xlo"] = np.ascontiguousarray(xlo[:, c * bl:(c + 1) * bl])
        in_maps.append(m)
    return in_maps


def kernel(x, W0, b0, W1, b1, W2, b2, Wc, bc, Wg, bg, trace=False):
    from concourse.bass_utils import run_bass_kernel_spmd

    if "nc" not in _CACHE:
        _CACHE["nc"] = build_program()
    nc = _CACHE["nc"]

    in_maps = _host_prep(x, W0, b0, W1, b1, W2, b2, Wc, bc, Wg, bg)
    res = run_bass_kernel_spmd(nc, in_maps, core_ids=list(range(NCORES)),
                               trace=trace)
    z = np.concatenate(
        [np.ascontiguousarray(res.results[c]["z"].T) for c in range(NCORES)],
        axis=0)
    y = np.concatenate([res.results[c]["y"] for c in range(NCORES)], axis=0)
    if trace:
        kernel.last_results = res
    return z, y.astype(np.int32)


kernel.last_results = None
